# revision 1
# baseline (speedup 1.0000x reference)
"""HSTU block kernel for Trainium2, 8-core data-parallel over batch.

Layouts are chosen so no on-device transposes are needed:
  - x is shipped both as xT [D, N] (for stats + as matmul rhs) and row-major
    (for the residual add).
  - proj is produced transposed (projT [E, N]) for u/q/k; v is produced
    row-major [N, DV*H] so it can be the stationary operand of the attn@v
    matmul.
  - qk logits are produced transposed (LT [key m, query n]); the rel-bias is
    accumulated in the same [m, n] layout and preloaded into PSUM via an
    identity matmul so the qk matmul accumulates on top of it.
  - ts_w[bucket(log(dt))] is piecewise-constant in log(dt): reconstructed with
    threshold-compare/accumulate passes (thresholds/coefs baked as immediates
    at build time; per-chunk threshold ranges pruned using the actual
    timestamp ranges, unioned across the 8 batches so one SPMD program works
    for all cores).
"""

import sys

sys.path.insert(0, "/opt/trn_rl_repo")

import numpy as np

import concourse.bass as bass
import concourse.tile as tile
import concourse.mybir as mybir
from concourse import bacc
from concourse.masks import make_identity

B, N, D = 8, 1024, 512
H, DV, DQ = 8, 64, 64
E = 2 * H * DV + 2 * H * DQ  # 2048
EPS = 1e-5
P = 128
NT = N // P  # 8 row tiles
F32 = mybir.dt.float32
F16 = mybir.dt.float16

_cache = {}


def _bucket(d):
    d = np.maximum(np.abs(d), 1).astype(np.float32)
    return np.clip((np.log(d) / 0.301).astype(np.int32), 0, 128)


def _plan_chunks(ts, tsq):
    """Uniform-across-batch k-ranges for the threshold passes."""
    far = []  # (r, n0, n1, kmin, kmax)
    for r in range(NT):
        n0 = P * (r + 2)
        while n0 < N:
            n1 = min(((n0 // 512) + 1) * 512, N)
            dmin = int((tsq[:, n0] - ts[:, P * r + P - 1]).min())
            dmax = int((tsq[:, n1 - 1] - ts[:, P * r]).max())
            far.append((r, n0, n1, int(_bucket(dmin)), int(_bucket(dmax))))
            n0 = n1
    # diag band: n in [128r, 128r+128), cells n >= m only
    dmin_g = int((tsq - ts).min())
    dmax_g = 0
    for r in range(NT):
        dmax_g = max(dmax_g, int((tsq[:, P * r + P - 1] - ts[:, P * r]).max()))
    kmin_g, kmax_g = int(_bucket(max(dmin_g, 0))), int(_bucket(dmax_g))
    # band1: n in [128(r+1), 128(r+2)) for r=0..6
    d1min = min(int((tsq[:, P * (r + 1)] - ts[:, P * r + P - 1]).min()) for r in range(NT - 1))
    d1max = max(int((tsq[:, P * (r + 2) - 1] - ts[:, P * r]).max()) for r in range(NT - 1))
    k1min, k1max = int(_bucket(max(d1min, 0))), int(_bucket(d1max))
    return far, kmin_g, kmax_g, k1min, k1max


def _build(ts_w_np, far, kmin_g, kmax_g, k1min, k1max):
    nc = bacc.Bacc()
    d = {}
    for name, shape in [
        ("xT", [D, N]), ("xr", [N, D]), ("tsq_rep", [P, N]), ("tsk_col", [P, NT]),
        ("uvqk_g", [D, E]), ("bU_col", [P, E // P]), ("bUv_rep", [P, DV * H]),
        ("W_o", [D, D]), ("b_o_row", [1, D]), ("ga_col", [P, 4]), ("bb_col", [P, 4]),
        ("vscale_col", [P, NT]), ("padout_col", [P, NT]), ("posacc", [P, 4608]),
    ]:
        d[name] = nc.dram_tensor(name, shape, F32, kind="ExternalInput")
    out_t = nc.dram_tensor("out", [N, D], F32, kind="ExternalOutput")

    widths = [N - P * r for r in range(NT)]
    offs = np.concatenate([[0], np.cumsum(widths)]).astype(int)
    tsw = ts_w_np.astype(np.float64)
    cks = [float(tsw[k] - tsw[k - 1]) for k in range(1, 129)]
    TH = 2.0 * 0.301  # y' = ln(d^2) threshold scale

    from contextlib import ExitStack
    with tile.TileContext(nc) as tc, ExitStack() as ctx:
        io = ctx.enter_context(tc.tile_pool(name="io", bufs=1))
        pools = ctx.enter_context(tc.tile_pool(name="work", bufs=4))
        kpool = ctx.enter_context(tc.tile_pool(name="kpool", bufs=2))
        psum = ctx.enter_context(tc.tile_pool(name="psum", bufs=2, space="PSUM"))
        psqk = ctx.enter_context(tc.tile_pool(name="psqk", bufs=2, space="PSUM"))
        psmall = ctx.enter_context(tc.tile_pool(name="psmall", bufs=2, space="PSUM"))

        # ---- persistent SBUF tensors ----
        xT = [io.tile([P, N], F32, tag=f"xT{s}", name=f"xT{s}") for s in range(4)]
        for s in range(4):
            nc.sync.dma_start(xT[s][:], d["xT"][P * s:P * s + P, :])
        wo = [io.tile([P, D], F32, tag=f"wo{s}", name=f"wo{s}") for s in range(4)]
        for s in range(4):
            nc.sync.dma_start(wo[s][:], d["W_o"][P * s:P * s + P, :])
        tsq_rep = io.tile([P, N], F32, tag="tsqr")
        nc.sync.dma_start(tsq_rep[:], d["tsq_rep"][:])
        small = {}
        for nm, sh in [("tsk_col", [P, NT]), ("bU_col", [P, E // P]),
                       ("bUv_rep", [P, DV * H]), ("b_o_row", [1, D]),
                       ("ga_col", [P, 4]), ("bb_col", [P, 4]),
                       ("vscale_col", [P, NT]), ("padout_col", [P, NT])]:
            small[nm] = io.tile(sh, F32, tag=nm, name=nm)
            nc.sync.dma_start(small[nm][:], d[nm][:])
        acc = [io.tile([P, widths[r]], F32, tag=f"acc{r}", name=f"acc{r}") for r in range(NT)]
        for r in range(NT):
            nc.sync.dma_start(acc[r][:], d["posacc"][:, offs[r]:offs[r + 1]])

        ident = io.tile([P, P], F32, tag="ident")
        make_identity(nc, ident[:])
        ones_col = io.tile([P, 1], F32, tag="ones_col")
        nc.vector.memset(ones_col[:], 1.0)
        ones_row = io.tile([1, P], F32, tag="ones_row")
        nc.vector.memset(ones_row[:], 1.0)

        # ---- layernorm stats of x (over D, via ones-matmul on xT) ----
        s1p = [psmall.tile([1, 512], F32, tag="s1", name=f"s1p{c}") for c in range(2)]
        s2p = [psmall.tile([1, 512], F32, tag="s2", name=f"s2p{c}") for c in range(2)]
        for s in range(4):
            sq = pools.tile([P, N], F32, tag="w32", name="sq")
            nc.vector.tensor_tensor(sq[:], xT[s][:], xT[s][:], mybir.AluOpType.mult)
            for c in range(2):
                nc.tensor.matmul(s1p[c][:], ones_col[:],
                                 xT[s][:, 512 * c:512 * c + 512],
                                 start=(s == 0), stop=(s == 3))
                nc.tensor.matmul(s2p[c][:], ones_col[:],
                                 sq[:, 512 * c:512 * c + 512],
                                 start=(s == 0), stop=(s == 3))
        mu = io.tile([1, N], F32, tag="mu")
        rs = io.tile([1, N], F32, tag="rs")
        tmp1 = pools.tile([1, N], F32, tag="w32", name="tmp1")
        for c in range(2):
            nc.vector.tensor_scalar_mul(mu[:, 512 * c:512 * c + 512], s1p[c][:], 1.0 / D)
            nc.vector.tensor_scalar_mul(tmp1[:, 512 * c:512 * c + 512], s2p[c][:], 1.0 / D)
        mu2 = pools.tile([1, N], F32, tag="w32", name="mu2")
        nc.vector.tensor_tensor(mu2[:], mu[:], mu[:], mybir.AluOpType.mult)
        nc.vector.tensor_tensor(tmp1[:], tmp1[:], mu2[:], mybir.AluOpType.subtract)
        nc.vector.tensor_scalar_add(tmp1[:], tmp1[:], EPS)
        nc.scalar.activation(tmp1[:], tmp1[:], mybir.ActivationFunctionType.Sqrt)
        nc.vector.reciprocal(rs[:], tmp1[:])

        # replicate mu, rs to [P, N]
        mur = io.tile([P, N], F32, tag="mur")
        rsr = io.tile([P, N], F32, tag="rsr")
        for vec, rep in [(mu, mur), (rs, rsr)]:
            for c in range(2):
                pt = psum.tile([P, 512], F32, tag="proj", name="rep")
                nc.tensor.matmul(pt[:], ones_row[:], vec[:, 512 * c:512 * c + 512],
                                 start=True, stop=True)
                nc.scalar.copy(out=rep[:, 512 * c:512 * c + 512], in_=pt[:])

        # xn'T = (xT - mu) * rs  (in place)
        xnt = xT
        for s in range(4):
            nc.vector.tensor_tensor(xnt[s][:], xT[s][:], mur[:], mybir.AluOpType.subtract)
            nc.vector.tensor_tensor(xnt[s][:], xnt[s][:], rsr[:], mybir.AluOpType.mult)

        # ---- projT for u,q,k tiles; v row-major ----
        uqk_tiles = [0, 1, 2, 3] + list(range(8, 16))
        projT = {}
        for t in uqk_tiles:
            projT[t] = io.tile([P, N], F16, tag=f"pT{t}", name=f"pT{t}")
            uvs = []
            for s in range(4):
                u1 = pools.tile([P, P], F32, tag="uvs", name="u1")
                nc.sync.dma_start(u1[:], d["uvqk_g"][P * s:P * s + P, P * t:P * t + P])
                uvs.append(u1)
            for c in range(2):
                pt = psum.tile([P, 512], F32, tag="proj")
                for s in range(4):
                    nc.tensor.matmul(pt[:], uvs[s][:],
                                     xnt[s][:, 512 * c:512 * c + 512],
                                     start=(s == 0), stop=(s == 3))
                nc.scalar.activation(projT[t][:, 512 * c:512 * c + 512], pt[:],
                                     mybir.ActivationFunctionType.Silu,
                                     bias=small["bU_col"][:, t:t + 1], scale=1.0)
        vt = [io.tile([P, D], F16, tag=f"v{r}", name=f"v{r}") for r in range(NT)]
        uvv = []
        for s in range(4):
            u2 = pools.tile([P, 512], F32, tag="uvv", name="u2")
            nc.sync.dma_start(u2[:], d["uvqk_g"][P * s:P * s + P, 512:1024])
            uvv.append(u2)
        for r in range(NT):
            pt = psum.tile([P, 512], F32, tag="proj")
            for s in range(4):
                nc.tensor.matmul(pt[:], xnt[s][:, P * r:P * r + P],
                                 uvv[s][:], start=(s == 0), stop=(s == 3))
            tmpv = pools.tile([P, D], F32, tag="w32", name="tmpv")
            nc.vector.tensor_tensor(tmpv[:], pt[:], small["bUv_rep"][:],
                                    mybir.AluOpType.add)
            nc.scalar.activation(tmpv[:], tmpv[:], mybir.ActivationFunctionType.Silu)
            nc.vector.tensor_scalar(vt[r][:], tmpv[:], small["vscale_col"][:, r:r + 1],
                                    None, mybir.AluOpType.mult)

        # ---- rel-bias threshold passes ----
        yh = [io.tile([P, widths[r]], F16, tag=f"yh{r}", name=f"yh{r}") for r in range(NT)]
        ystack = io.tile([P, N], F16, tag="ystack")
        ystack2 = io.tile([P, N - P], F16, tag="ystack2")
        dacc2 = io.tile([P, N - P], F16, tag="dacc2")
        nc.vector.memset(dacc2[:], 0.0)
        acch = [io.tile([P, widths[r]], F16, tag=f"acch{r}", name=f"acch{r}") for r in range(NT)]
        dacc = io.tile([P, N], F16, tag="dacc")
        nc.vector.memset(dacc[:], 0.0)
        for r in range(NT):
            w = widths[r]
            nc.vector.memset(acch[r][:], 0.0)
            db = pools.tile([P, N], F32, tag="w32", name="db")
            d2 = pools.tile([P, N], F32, tag="w32", name="d2")
            nc.vector.tensor_scalar(db[:, :w], tsq_rep[:, P * r:N],
                                    small["tsk_col"][:, r:r + 1], None,
                                    mybir.AluOpType.subtract)
            nc.vector.tensor_tensor(d2[:, :w], db[:, :w], db[:, :w],
                                    mybir.AluOpType.mult)
            nc.scalar.activation(db[:, :w], d2[:, :w],
                                 mybir.ActivationFunctionType.Ln)
            nc.vector.tensor_copy(out=yh[r][:], in_=db[:, :w])
            nc.vector.tensor_copy(out=ystack[:, P * r:P * r + P], in_=yh[r][:, 0:P])
            if r < NT - 1:
                nc.vector.tensor_copy(out=ystack2[:, P * r:P * r + P], in_=yh[r][:, P:2 * P])
        # diag band passes (shared stack, one instr per k); top of the
        # k-range runs on GPSIMD (fp32) to overlap with the DVE chain
        ksplit = kmax_g - max(1, (kmax_g - kmin_g) * 2 // 5)
        ystack32 = io.tile([P, N], F32, tag="rsr", name="ystack32")
        nc.gpsimd.tensor_copy(out=ystack32[:], in_=ystack[:])
        gacc = io.tile([P, N], F32, tag="mur", name="gacc")
        nc.gpsimd.memset(gacc[:], 0.0)
        for k in range(kmin_g + 1, ksplit + 1):
            t = kpool.tile([P, N], F16, tag="kt")
            nc.vector.tensor_scalar(t[:], ystack[:], float(TH * k), cks[k - 1],
                                    mybir.AluOpType.is_ge, mybir.AluOpType.mult)
            nc.vector.tensor_tensor(dacc[:], dacc[:], t[:], mybir.AluOpType.add)
        for k in range(ksplit + 1, kmax_g + 1):
            tg = kpool.tile([P, N], F32, tag="ktg")
            nc.gpsimd.tensor_scalar(tg[:], ystack32[:], float(TH * k), cks[k - 1],
                                    mybir.AluOpType.is_ge, mybir.AluOpType.mult)
            nc.gpsimd.tensor_tensor(gacc[:], gacc[:], tg[:], mybir.AluOpType.add)
        # band1 passes
        for k in range(k1min + 1, k1max + 1):
            t = kpool.tile([P, N], F16, tag="kt")
            nc.vector.tensor_scalar(t[:, :N - P], ystack2[:], float(TH * k), cks[k - 1],
                                    mybir.AluOpType.is_ge, mybir.AluOpType.mult)
            nc.vector.tensor_tensor(dacc2[:], dacc2[:], t[:, :N - P], mybir.AluOpType.add)
        # far chunk passes
        for (r, n0, n1, kmin, kmax) in far:
            a, b2 = n0 - P * r, n1 - P * r
            for k in range(kmin + 1, kmax + 1):
                t = kpool.tile([P, N], F16, tag="kt")
                nc.vector.tensor_scalar(t[:, :b2 - a], yh[r][:, a:b2], float(TH * k),
                                        cks[k - 1], mybir.AluOpType.is_ge,
                                        mybir.AluOpType.mult)
                nc.vector.tensor_tensor(acch[r][:, a:b2], acch[r][:, a:b2],
                                        t[:, :b2 - a], mybir.AluOpType.add)
        for r in range(NT):
            cf = pools.tile([P, N], F32, tag="w32", name="cf")
            nc.vector.tensor_copy(out=cf[:, :widths[r]], in_=acch[r][:])
            nc.vector.tensor_tensor(acc[r][:], acc[r][:], cf[:, :widths[r]],
                                    mybir.AluOpType.add)
            cf2 = pools.tile([P, P], F32, tag="w32", name="cf2")
            nc.vector.tensor_copy(out=cf2[:], in_=dacc[:, P * r:P * r + P])
            nc.vector.tensor_tensor(acc[r][:, 0:P], acc[r][:, 0:P], cf2[:],
                                    mybir.AluOpType.add)
            nc.vector.tensor_tensor(acc[r][:, 0:P], acc[r][:, 0:P],
                                    gacc[:, P * r:P * r + P], mybir.AluOpType.add)
            if r < NT - 1:
                cf3 = pools.tile([P, P], F32, tag="w32", name="cf3")
                nc.vector.tensor_copy(out=cf3[:], in_=dacc2[:, P * r:P * r + P])
                nc.vector.tensor_tensor(acc[r][:, P:2 * P], acc[r][:, P:2 * P], cf3[:],
                                        mybir.AluOpType.add)

        # ---- attention per head ----
        qksil = [io.tile([P, N], F16, tag=f"qs{r}", name=f"qs{r}") for r in range(NT)]
        for r in range(NT):
            nc.vector.memset(qksil[r][:], 0.0)
        attnT = [io.tile([P, N], F32, tag=f"aT{t}", name=f"aT{t}") for t in range(4)]
        for h in range(H):
            qt = projT[8 + h // 2]
            kt = projT[12 + h // 2]
            pq = 64 * (h % 2)
            for r in range(NT):
                n0 = P * r
                while n0 < N:
                    n1 = min(((n0 // 512) + 1) * 512, N)
                    pt = psqk.tile([P, 512], F32, tag="qk")
                    cw = n1 - n0
                    nc.tensor.matmul(pt[:, :cw], ident[:],
                                     acc[r][:, n0 - P * r:n1 - P * r],
                                     start=True, stop=False)
                    nc.tensor.matmul(pt[:, :cw], kt[pq:pq + 64, P * r:P * r + P],
                                     qt[pq:pq + 64, n0:n1], start=False, stop=True)
                    nc.scalar.activation(qksil[r][:, n0:n1], pt[:, :cw],
                                         mybir.ActivationFunctionType.Silu)
                    n0 = n1
                nc.gpsimd.affine_select(
                    out=qksil[r][:, P * r:P * r + P], in_=qksil[r][:, P * r:P * r + P],
                    pattern=[[1, P]], compare_op=mybir.AluOpType.is_ge, fill=0.0,
                    base=0, channel_multiplier=-1)
            for c in range(2):
                pa = psqk.tile([P, 512], F32, tag="qk", name="av")
                nsub = min(NT, 4 * (c + 1))
                for r in range(nsub):
                    nc.tensor.matmul(pa[:64, :], vt[r][:, 64 * h:64 * h + 64],
                                     qksil[r][:, 512 * c:512 * c + 512],
                                     start=(r == 0), stop=(r == nsub - 1))
                at = attnT[h // 2]
                nc.scalar.copy(out=at[pq:pq + 64, 512 * c:512 * c + 512],
                               in_=pa[:64, :])

        # ---- layernorm of attn (over E=512, partition dim) ----
        sa1 = [psmall.tile([1, 512], F32, tag="s1", name=f"sa1{c}") for c in range(2)]
        sa2 = [psmall.tile([1, 512], F32, tag="s2", name=f"sa2{c}") for c in range(2)]
        for c in range(2):
            for s in range(4):
                nc.tensor.matmul(sa1[c][:], ones_col[:],
                                 attnT[s][:, 512 * c:512 * c + 512],
                                 start=(s == 0), stop=(s == 3))
            for s in range(4):
                sqa = pools.tile([P, 512], F32, tag="w32", name="sqa")
                nc.vector.tensor_tensor(sqa[:], attnT[s][:, 512 * c:512 * c + 512],
                                        attnT[s][:, 512 * c:512 * c + 512],
                                        mybir.AluOpType.mult)
                nc.tensor.matmul(sa2[c][:], ones_col[:], sqa[:],
                                 start=(s == 0), stop=(s == 3))
        mua = io.tile([1, N], F32, tag="mua")
        rsa = io.tile([1, N], F32, tag="rsa")
        tmpa = pools.tile([1, N], F32, tag="w32", name="tmpa")
        for c in range(2):
            nc.vector.tensor_scalar_mul(mua[:, 512 * c:512 * c + 512], sa1[c][:], 1.0 / D)
            nc.vector.tensor_scalar_mul(tmpa[:, 512 * c:512 * c + 512], sa2[c][:], 1.0 / D)
        mua2 = pools.tile([1, N], F32, tag="w32", name="mua2")
        nc.vector.tensor_tensor(mua2[:], mua[:], mua[:], mybir.AluOpType.mult)
        nc.vector.tensor_tensor(tmpa[:], tmpa[:], mua2[:], mybir.AluOpType.subtract)
        nc.vector.tensor_scalar_add(tmpa[:], tmpa[:], EPS)
        nc.scalar.activation(tmpa[:], tmpa[:], mybir.ActivationFunctionType.Sqrt)
        nc.vector.reciprocal(rsa[:], tmpa[:])
        muar = io.tile([P, N], F32, tag="mur", name="muar")
        rsar = io.tile([P, N], F32, tag="rsr", name="rsar")
        for vec, rep in [(mua, muar), (rsa, rsar)]:
            for c in range(2):
                pt = psum.tile([P, 512], F32, tag="proj", name="rep")
                nc.tensor.matmul(pt[:], ones_row[:], vec[:, 512 * c:512 * c + 512],
                                 start=True, stop=True)
                nc.scalar.copy(out=rep[:, 512 * c:512 * c + 512], in_=pt[:])
        # prod = u * (LN_a(attn)*gamma+beta), in attnT layout
        for s in range(4):
            nc.vector.tensor_tensor(attnT[s][:], attnT[s][:], muar[:],
                                    mybir.AluOpType.subtract)
            nc.vector.tensor_tensor(attnT[s][:], attnT[s][:], rsar[:],
                                    mybir.AluOpType.mult)
            nc.vector.tensor_scalar(attnT[s][:], attnT[s][:],
                                    small["ga_col"][:, s:s + 1],
                                    small["bb_col"][:, s:s + 1],
                                    mybir.AluOpType.mult, mybir.AluOpType.add)
            nc.vector.tensor_tensor(attnT[s][:], attnT[s][:], projT[s][:],
                                    mybir.AluOpType.mult)

        # ---- output projection + residual ----
        for t in range(NT):
            po = psum.tile([P, 512], F32, tag="proj", name="outp")
            for s in range(4):
                nc.tensor.matmul(po[:], attnT[s][:, P * t:P * t + P], wo[s][:],
                                 start=(s == 0), stop=False)
            nc.tensor.matmul(po[:], ones_row[:], small["b_o_row"][:],
                             start=False, stop=True)
            xtile = pools.tile([P, D], F32, tag="w32", name="xtile")
            nc.sync.dma_start(xtile[:], d["xr"][P * t:P * t + P, :])
            ot = pools.tile([P, D], F32, tag="w32", name="ot")
            nc.vector.tensor_tensor(ot[:], po[:], xtile[:], mybir.AluOpType.add)
            nc.vector.tensor_scalar(ot[:], ot[:], small["padout_col"][:, t:t + 1],
                                    None, mybir.AluOpType.mult)
            nc.sync.dma_start(out_t[P * t:P * t + P, :], ot[:])

    nc.compile()
    return nc


def _prep_inputs(inputs):
    x = np.asarray(inputs["x"], dtype=np.float32)
    ts = np.asarray(inputs["timestamps"]).astype(np.int64)
    pad = np.asarray(inputs["pad_mask"]).astype(np.float32)
    uvqk = np.asarray(inputs["uvqk"], dtype=np.float32)
    W_o = np.asarray(inputs["W_o"], dtype=np.float32)
    b_o = np.asarray(inputs["b_o"], dtype=np.float32)
    gx = np.asarray(inputs["gamma_x"], dtype=np.float32)
    bx = np.asarray(inputs["beta_x"], dtype=np.float32)
    ga = np.asarray(inputs["gamma_a"], dtype=np.float32)
    ba = np.asarray(inputs["beta_a"], dtype=np.float32)
    ts_w = np.asarray(inputs["ts_w"], dtype=np.float32)
    pos_w = np.asarray(inputs["pos_w"], dtype=np.float32)

    tsq = np.concatenate([ts[:, 1:], ts[:, -1:]], axis=1)  # [B, N]
    far, kmin_g, kmax_g, k1min, k1max = _plan_chunks(ts, tsq)

    uvqk_g = uvqk * gx[:, None]
    bU = bx @ uvqk  # [E]
    bU_col = bU.reshape(E // P, P).T.copy()  # [P, E//P]
    bUv_rep = np.broadcast_to(bU[512:1024], (P, 512)).copy()
    ga_col = ga.reshape(4, P).T.copy()
    ba_col = ba.reshape(4, P).T.copy()

    # pos-bias tiles in [m, n] layout + per-chunk base constants
    widths = [N - P * r for r in range(NT)]
    offs = np.concatenate([[0], np.cumsum(widths)]).astype(int)
    posacc = np.zeros((P, int(offs[-1])), np.float32)
    nidx = np.arange(N)
    for r in range(NT):
        m = P * r + np.arange(P)[:, None]
        nn = nidx[None, P * r:]
        posacc[:, offs[r]:offs[r + 1]] = pos_w[nn - m + (N - 1)]
        posacc[:, offs[r]:offs[r] + P] += ts_w[kmin_g]
        if r < NT - 1:
            posacc[:, offs[r] + P:offs[r] + 2 * P] += ts_w[k1min]
    for (r, n0, n1, kmin, kmax) in far:
        posacc[:, offs[r] + n0 - P * r: offs[r] + n1 - P * r] += ts_w[kmin]

    per_core = []
    for b in range(B):
        per_core.append({
            "xT": np.ascontiguousarray(x[b].T),
            "xr": np.ascontiguousarray(x[b]),
            "tsq_rep": np.broadcast_to(tsq[b].astype(np.float32), (P, N)).copy(),
            "tsk_col": np.ascontiguousarray(ts[b].astype(np.float32).reshape(NT, P).T),
            "uvqk_g": uvqk_g, "bU_col": bU_col, "bUv_rep": bUv_rep,
            "W_o": W_o, "b_o_row": b_o.reshape(1, D),
            "ga_col": ga_col, "bb_col": ba_col,
            "vscale_col": np.ascontiguousarray(
                ((1.0 - pad[b]) / N).astype(np.float32).reshape(NT, P).T),
            "padout_col": np.ascontiguousarray(
                (1.0 - pad[b]).astype(np.float32).reshape(NT, P).T),
            "posacc": posacc,
        })
    return per_core, (far, kmin_g, kmax_g, k1min, k1max, ts_w)


def kernel(**inputs):
    from concourse.bass_utils import run_bass_kernel_spmd

    per_core, (far, kmin_g, kmax_g, k1min, k1max, ts_w) = _prep_inputs(inputs)
    key = (tuple(far), kmin_g, kmax_g, k1min, k1max, ts_w.tobytes())
    if key not in _cache:
        _cache.clear()
        _cache[key] = _build(ts_w, far, kmin_g, kmax_g, k1min, k1max)
    nc = _cache[key]
    res = run_bass_kernel_spmd(nc, per_core, list(range(B)))
    out = np.stack([res.results[b]["out"] for b in range(B)], axis=0)
    return out.astype(np.float32)



# revision 15
# speedup vs baseline: 1.4196x; 1.4196x over previous
"""HSTU block kernel for Trainium2, 8-core data-parallel over batch.

Layouts are chosen so no on-device transposes are needed:
  - x is shipped both as xT [D, N] in bf16 (for stats + as matmul rhs) and
    row-major f32 (for the residual add).
  - All PE matmul operands are 16-bit (bf16/f16): 1 cycle/row instead of 4
    for fp32, with fp32 PSUM accumulation.
  - proj is produced transposed (projT [E, N]) for u/q/k; v is produced
    row-major [N, DV*H] so it can be the stationary operand of the attn@v
    matmul.
  - qk logits are produced transposed (LT [key m, query n]); the rel-bias is
    accumulated in the same [m, n] layout (f16) and preloaded into PSUM via an
    identity matmul so the qk matmul accumulates on top of it.
  - ts_w[bucket(log(dt))] is piecewise-constant in log(dt): reconstructed with
    threshold-compare/accumulate passes (thresholds/coefs baked as immediates
    at build time; per-chunk threshold ranges pruned using the actual
    timestamp ranges, unioned across the 8 batches so one SPMD program works
    for all cores).
"""

import sys

sys.path.insert(0, "/opt/trn_rl_repo")

import numpy as np
import ml_dtypes

import concourse.bass as bass
import concourse.tile as tile
import concourse.mybir as mybir
from concourse import bacc
from concourse.masks import make_identity

B, N, D = 8, 1024, 512
H, DV, DQ = 8, 64, 64
E = 2 * H * DV + 2 * H * DQ  # 2048
EPS = 1e-5
P = 128
NT = N // P  # 8 row tiles
F32 = mybir.dt.float32
F16 = mybir.dt.float16
BF16 = mybir.dt.bfloat16
NPBF = np.dtype(ml_dtypes.bfloat16)

_cache = {}


def _bucket(d):
    d = np.maximum(np.abs(d), 1).astype(np.float32)
    return np.clip((np.log(d) / 0.301).astype(np.int32), 0, 128)


def _plan_chunks(ts, tsq):
    """Uniform-across-batch k-ranges for the threshold passes."""
    far = []  # (r, n0, n1, kmin, kmax)
    for r in range(NT):
        n0 = P * (r + 2)
        while n0 < N:
            n1 = min(((n0 // 512) + 1) * 512, N)
            dmin = int((tsq[:, n0] - ts[:, P * r + P - 1]).min())
            dmax = int((tsq[:, n1 - 1] - ts[:, P * r]).max())
            far.append((r, n0, n1, int(_bucket(dmin)), int(_bucket(dmax))))
            n0 = n1
    # diag band: n in [128r, 128r+128), cells n >= m only
    dmin_g = int((tsq - ts).min())
    dmax_g = 0
    for r in range(NT):
        dmax_g = max(dmax_g, int((tsq[:, P * r + P - 1] - ts[:, P * r]).max()))
    kmin_g, kmax_g = int(_bucket(max(dmin_g, 0))), int(_bucket(dmax_g))
    # band1: n in [128(r+1), 128(r+2)) for r=0..6
    d1min = min(int((tsq[:, P * (r + 1)] - ts[:, P * r + P - 1]).min()) for r in range(NT - 1))
    d1max = max(int((tsq[:, P * (r + 2) - 1] - ts[:, P * r]).max()) for r in range(NT - 1))
    k1min, k1max = int(_bucket(max(d1min, 0))), int(_bucket(d1max))
    return far, kmin_g, kmax_g, k1min, k1max


def _build(ts_w_np, far, kmin_g, kmax_g, k1min, k1max):
    nc = bacc.Bacc()
    d = {}
    for name, shape, dt_ in [
        ("xT", [D, N], BF16), ("xr", [N, D], F32), ("tsq_rep", [P, N], F32),
        ("tsk_col", [P, NT], F32), ("uvqk_g", [D, E], BF16),
        ("bU_col", [P, E // P], F32), ("bUv_rep", [P, DV * H], F32),
        ("W_o", [D, D], BF16), ("b_o_row", [1, D], BF16),
        ("ga_col", [P, 4], F32), ("bb_col", [P, 4], F32),
        ("vscale_col", [P, NT], F32), ("padout_col", [P, NT], F32),
        ("posacc", [P, 4608], F16),
    ]:
        d[name] = nc.dram_tensor(name, shape, dt_, kind="ExternalInput")
    out_t = nc.dram_tensor("out", [N, D], F32, kind="ExternalOutput")

    widths = [N - P * r for r in range(NT)]
    offs = np.concatenate([[0], np.cumsum(widths)]).astype(int)
    tsw = ts_w_np.astype(np.float64)
    cks = [float(tsw[k] - tsw[k - 1]) for k in range(1, 129)]
    TH = 2.0 * 0.301  # y' = ln(d^2) threshold scale

    from contextlib import ExitStack
    with tile.TileContext(nc) as tc, ExitStack() as ctx:
        io = ctx.enter_context(tc.tile_pool(name="io", bufs=1))
        pools = ctx.enter_context(tc.tile_pool(name="work", bufs=4))
        kpool = ctx.enter_context(tc.tile_pool(name="kpool", bufs=2))
        psum = ctx.enter_context(tc.tile_pool(name="psum", bufs=2, space="PSUM"))
        psqk = ctx.enter_context(tc.tile_pool(name="psqk", bufs=2, space="PSUM"))
        psmall = ctx.enter_context(tc.tile_pool(name="psmall", bufs=2, space="PSUM"))

        # ---- persistent SBUF tensors ----
        xT = [io.tile([P, N], BF16, tag=f"xT{s}", name=f"xT{s}") for s in range(4)]
        for s in range(4):
            nc.sync.dma_start(xT[s][:], d["xT"][P * s:P * s + P, :])
        wo = [io.tile([P, D], BF16, tag=f"wo{s}", name=f"wo{s}") for s in range(4)]
        for s in range(4):
            nc.sync.dma_start(wo[s][:], d["W_o"][P * s:P * s + P, :])
        tsq_rep = io.tile([P, N], F32, tag="tsqr")
        nc.sync.dma_start(tsq_rep[:], d["tsq_rep"][:])
        small = {}
        for nm, sh, dt_ in [("tsk_col", [P, NT], F32), ("bU_col", [P, E // P], F32),
                            ("bUv_rep", [P, DV * H], F32), ("b_o_row", [1, D], BF16),
                            ("ga_col", [P, 4], F32), ("bb_col", [P, 4], F32),
                            ("vscale_col", [P, NT], F32), ("padout_col", [P, NT], F32)]:
            small[nm] = io.tile(sh, dt_, tag=nm, name=nm)
            nc.sync.dma_start(small[nm][:], d[nm][:])
        acc = [io.tile([P, widths[r]], F16, tag=f"acc{r}", name=f"acc{r}") for r in range(NT)]
        for r in range(NT):
            nc.sync.dma_start(acc[r][:], d["posacc"][:, offs[r]:offs[r + 1]])

        ident = io.tile([P, P], F16, tag="ident")
        make_identity(nc, ident[:])
        ones_col = io.tile([P, 1], BF16, tag="ones_col")
        nc.vector.memset(ones_col[:], 1.0)
        ones_row = io.tile([1, P], BF16, tag="ones_row")
        nc.vector.memset(ones_row[:], 1.0)

        # ---- layernorm stats of x (over D, via ones-matmul on xT) ----
        s1p = [psmall.tile([1, 512], F32, tag="s1", name=f"s1p{c}") for c in range(2)]
        s2p = [psmall.tile([1, 512], F32, tag="s2", name=f"s2p{c}") for c in range(2)]
        for s in range(4):
            sq = pools.tile([P, N], BF16, tag="wb16", name="sq")
            nc.vector.tensor_tensor(sq[:], xT[s][:], xT[s][:], mybir.AluOpType.mult)
            for c in range(2):
                nc.tensor.matmul(s1p[c][:], ones_col[:],
                                 xT[s][:, 512 * c:512 * c + 512],
                                 start=(s == 0), stop=(s == 3))
                nc.tensor.matmul(s2p[c][:], ones_col[:],
                                 sq[:, 512 * c:512 * c + 512],
                                 start=(s == 0), stop=(s == 3))
        mu = io.tile([1, N], F32, tag="mu")
        rs = io.tile([1, N], F32, tag="rs")
        tmp1 = pools.tile([1, N], F32, tag="w32", name="tmp1")
        for c in range(2):
            nc.vector.tensor_scalar_mul(mu[:, 512 * c:512 * c + 512], s1p[c][:], 1.0 / D)
            nc.vector.tensor_scalar_mul(tmp1[:, 512 * c:512 * c + 512], s2p[c][:], 1.0 / D)
        mu2 = pools.tile([1, N], F32, tag="w32", name="mu2")
        nc.vector.tensor_tensor(mu2[:], mu[:], mu[:], mybir.AluOpType.mult)
        nc.vector.tensor_tensor(tmp1[:], tmp1[:], mu2[:], mybir.AluOpType.subtract)
        nc.vector.tensor_scalar_add(tmp1[:], tmp1[:], EPS)
        nc.scalar.activation(tmp1[:], tmp1[:], mybir.ActivationFunctionType.Sqrt)
        nc.vector.reciprocal(rs[:], tmp1[:])
        mu16 = io.tile([1, N], BF16, tag="mu16")
        rs16 = io.tile([1, N], BF16, tag="rs16")
        nc.scalar.copy(out=mu16[:], in_=mu[:])
        nc.scalar.copy(out=rs16[:], in_=rs[:])

        # replicate mu, rs to [P, N] (bf16)
        mur = io.tile([P, N], BF16, tag="mur")
        rsr = io.tile([P, N], BF16, tag="rsr")
        for vec, rep in [(mu16, mur), (rs16, rsr)]:
            for c in range(2):
                pt = psum.tile([P, 512], F32, tag="proj", name="rep")
                nc.tensor.matmul(pt[:], ones_row[:], vec[:, 512 * c:512 * c + 512],
                                 start=True, stop=True)
                nc.scalar.copy(out=rep[:, 512 * c:512 * c + 512], in_=pt[:])

        # xn'T = (xT - mu) * rs  (in place, bf16)
        xnt = xT
        for s in range(4):
            nc.vector.tensor_tensor(xnt[s][:], xT[s][:], mur[:], mybir.AluOpType.subtract)
            nc.vector.tensor_tensor(xnt[s][:], xnt[s][:], rsr[:], mybir.AluOpType.mult)

        # ---- projT for u,q,k tiles; v row-major ----
        uqk_tiles = [0, 1, 2, 3] + list(range(8, 16))
        projT = {}
        for t in uqk_tiles:
            pdt = BF16 if t < 4 else F16
            projT[t] = io.tile([P, N], pdt, tag=f"pT{t}", name=f"pT{t}")
            uvs = []
            for s in range(4):
                u1 = pools.tile([P, P], BF16, tag="uvs", name="u1")
                nc.sync.dma_start(u1[:], d["uvqk_g"][P * s:P * s + P, P * t:P * t + P])
                uvs.append(u1)
            for c in range(2):
                pt = psum.tile([P, 512], F32, tag="proj")
                for s in range(4):
                    nc.tensor.matmul(pt[:], uvs[s][:],
                                     xnt[s][:, 512 * c:512 * c + 512],
                                     start=(s == 0), stop=(s == 3))
                nc.scalar.activation(projT[t][:, 512 * c:512 * c + 512], pt[:],
                                     mybir.ActivationFunctionType.Silu,
                                     bias=small["bU_col"][:, t:t + 1], scale=1.0)
        vt = [io.tile([P, D], F16, tag=f"v{r}", name=f"v{r}") for r in range(NT)]
        uvv = []
        for s in range(4):
            u2 = pools.tile([P, 512], BF16, tag="uvv", name="u2")
            nc.sync.dma_start(u2[:], d["uvqk_g"][P * s:P * s + P, 512:1024])
            uvv.append(u2)
        for r in range(NT):
            pt = psum.tile([P, 512], F32, tag="proj")
            for s in range(4):
                nc.tensor.matmul(pt[:], xnt[s][:, P * r:P * r + P],
                                 uvv[s][:], start=(s == 0), stop=(s == 3))
            tmpv = pools.tile([P, D], F32, tag="w32", name="tmpv")
            nc.vector.tensor_tensor(tmpv[:], pt[:], small["bUv_rep"][:],
                                    mybir.AluOpType.add)
            nc.scalar.activation(tmpv[:], tmpv[:], mybir.ActivationFunctionType.Silu)
            nc.vector.tensor_scalar(vt[r][:], tmpv[:], small["vscale_col"][:, r:r + 1],
                                    None, mybir.AluOpType.mult)

        # ---- rel-bias threshold passes ----
        yh = [io.tile([P, widths[r]], F16, tag=f"yh{r}", name=f"yh{r}") for r in range(NT)]
        ystack = io.tile([P, N], F16, tag="ystack")
        ystack2 = io.tile([P, N - P], F16, tag="ystack2")
        dacc2 = io.tile([P, N - P], F16, tag="dacc2")
        nc.vector.memset(dacc2[:], 0.0)
        acch = [io.tile([P, widths[r]], F16, tag=f"acch{r}", name=f"acch{r}") for r in range(NT)]
        dacc = io.tile([P, N], F16, tag="dacc")
        nc.vector.memset(dacc[:], 0.0)
        for r in range(NT):
            w = widths[r]
            nc.vector.memset(acch[r][:], 0.0)
            db = pools.tile([P, N], F32, tag="w32", name="db")
            d2 = pools.tile([P, N], F32, tag="w32", name="d2")
            nc.vector.tensor_scalar(db[:, :w], tsq_rep[:, P * r:N],
                                    small["tsk_col"][:, r:r + 1], None,
                                    mybir.AluOpType.subtract)
            nc.vector.tensor_tensor(d2[:, :w], db[:, :w], db[:, :w],
                                    mybir.AluOpType.mult)
            nc.scalar.activation(db[:, :w], d2[:, :w],
                                 mybir.ActivationFunctionType.Ln)
            nc.vector.tensor_copy(out=yh[r][:], in_=db[:, :w])
            nc.vector.tensor_copy(out=ystack[:, P * r:P * r + P], in_=yh[r][:, 0:P])
            if r < NT - 1:
                nc.vector.tensor_copy(out=ystack2[:, P * r:P * r + P], in_=yh[r][:, P:2 * P])
        # diag band passes (shared stack, one instr per k); top of the
        # k-range runs on GPSIMD (fp32) to overlap with the DVE chain
        ksplit = kmax_g - max(1, (kmax_g - kmin_g) * 2 // 5)
        ystack32 = io.tile([P, N], F32, tag="ys32", name="ystack32")
        nc.gpsimd.tensor_copy(out=ystack32[:], in_=ystack[:])
        gacc = io.tile([P, N], F32, tag="gacc", name="gacc")
        nc.gpsimd.memset(gacc[:], 0.0)
        for k in range(kmin_g + 1, ksplit + 1):
            t = kpool.tile([P, N], F16, tag="kt")
            nc.vector.tensor_scalar(t[:], ystack[:], float(TH * k), cks[k - 1],
                                    mybir.AluOpType.is_ge, mybir.AluOpType.mult)
            nc.vector.tensor_tensor(dacc[:], dacc[:], t[:], mybir.AluOpType.add)
        for k in range(ksplit + 1, kmax_g + 1):
            tg = kpool.tile([P, N], F32, tag="ktg")
            nc.gpsimd.tensor_scalar(tg[:], ystack32[:], float(TH * k), cks[k - 1],
                                    mybir.AluOpType.is_ge, mybir.AluOpType.mult)
            nc.gpsimd.tensor_tensor(gacc[:], gacc[:], tg[:], mybir.AluOpType.add)
        # band1 passes
        for k in range(k1min + 1, k1max + 1):
            t = kpool.tile([P, N], F16, tag="kt")
            nc.vector.tensor_scalar(t[:, :N - P], ystack2[:], float(TH * k), cks[k - 1],
                                    mybir.AluOpType.is_ge, mybir.AluOpType.mult)
            nc.vector.tensor_tensor(dacc2[:], dacc2[:], t[:, :N - P], mybir.AluOpType.add)
        # far chunk passes
        for (r, n0, n1, kmin, kmax) in far:
            a, b2 = n0 - P * r, n1 - P * r
            for k in range(kmin + 1, kmax + 1):
                t = kpool.tile([P, N], F16, tag="kt")
                nc.vector.tensor_scalar(t[:, :b2 - a], yh[r][:, a:b2], float(TH * k),
                                        cks[k - 1], mybir.AluOpType.is_ge,
                                        mybir.AluOpType.mult)
                nc.vector.tensor_tensor(acch[r][:, a:b2], acch[r][:, a:b2],
                                        t[:, :b2 - a], mybir.AluOpType.add)
        gacc16 = io.tile([P, N], F16, tag="gacc16")
        nc.scalar.copy(out=gacc16[:], in_=gacc[:])
        for r in range(NT):
            nc.vector.tensor_tensor(acc[r][:], acc[r][:], acch[r][:],
                                    mybir.AluOpType.add)
            nc.vector.tensor_tensor(acc[r][:, 0:P], acc[r][:, 0:P],
                                    dacc[:, P * r:P * r + P], mybir.AluOpType.add)
            nc.vector.tensor_tensor(acc[r][:, 0:P], acc[r][:, 0:P],
                                    gacc16[:, P * r:P * r + P], mybir.AluOpType.add)
            if r < NT - 1:
                nc.vector.tensor_tensor(acc[r][:, P:2 * P], acc[r][:, P:2 * P],
                                        dacc2[:, P * r:P * r + P],
                                        mybir.AluOpType.add)

        # ---- attention per head ----
        qksil = [io.tile([P, N], F16, tag=f"qs{r}", name=f"qs{r}") for r in range(NT)]
        for r in range(NT):
            nc.vector.memset(qksil[r][:], 0.0)
        attnT = [io.tile([P, N], BF16, tag=f"aT{t}", name=f"aT{t}") for t in range(4)]
        for h in range(H):
            qt = projT[8 + h // 2]
            kt = projT[12 + h // 2]
            pq = 64 * (h % 2)
            for r in range(NT):
                n0 = P * r
                while n0 < N:
                    n1 = min(((n0 // 512) + 1) * 512, N)
                    pt = psqk.tile([P, 512], F32, tag="qk")
                    cw = n1 - n0
                    nc.tensor.matmul(pt[:, :cw], ident[:],
                                     acc[r][:, n0 - P * r:n1 - P * r],
                                     start=True, stop=False)
                    nc.tensor.matmul(pt[:, :cw], kt[pq:pq + 64, P * r:P * r + P],
                                     qt[pq:pq + 64, n0:n1], start=False, stop=True)
                    nc.scalar.activation(qksil[r][:, n0:n1], pt[:, :cw],
                                         mybir.ActivationFunctionType.Silu)
                    n0 = n1
                nc.gpsimd.affine_select(
                    out=qksil[r][:, P * r:P * r + P], in_=qksil[r][:, P * r:P * r + P],
                    pattern=[[1, P]], compare_op=mybir.AluOpType.is_ge, fill=0.0,
                    base=0, channel_multiplier=-1)
            for c in range(2):
                pa = psqk.tile([P, 512], F32, tag="qk", name="av")
                nsub = min(NT, 4 * (c + 1))
                for r in range(nsub):
                    nc.tensor.matmul(pa[:64, :], vt[r][:, 64 * h:64 * h + 64],
                                     qksil[r][:, 512 * c:512 * c + 512],
                                     start=(r == 0), stop=(r == nsub - 1))
                at = attnT[h // 2]
                nc.scalar.copy(out=at[pq:pq + 64, 512 * c:512 * c + 512],
                               in_=pa[:64, :])

        # ---- layernorm of attn (over E=512, partition dim) ----
        sa1 = [psmall.tile([1, 512], F32, tag="s1", name=f"sa1{c}") for c in range(2)]
        sa2 = [psmall.tile([1, 512], F32, tag="s2", name=f"sa2{c}") for c in range(2)]
        for c in range(2):
            for s in range(4):
                nc.tensor.matmul(sa1[c][:], ones_col[:],
                                 attnT[s][:, 512 * c:512 * c + 512],
                                 start=(s == 0), stop=(s == 3))
            for s in range(4):
                sqa = pools.tile([P, 512], BF16, tag="wb16", name="sqa")
                nc.vector.tensor_tensor(sqa[:], attnT[s][:, 512 * c:512 * c + 512],
                                        attnT[s][:, 512 * c:512 * c + 512],
                                        mybir.AluOpType.mult)
                nc.tensor.matmul(sa2[c][:], ones_col[:], sqa[:],
                                 start=(s == 0), stop=(s == 3))
        mua = io.tile([1, N], F32, tag="mua")
        rsa = io.tile([1, N], F32, tag="rsa")
        tmpa = pools.tile([1, N], F32, tag="w32", name="tmpa")
        for c in range(2):
            nc.vector.tensor_scalar_mul(mua[:, 512 * c:512 * c + 512], sa1[c][:], 1.0 / D)
            nc.vector.tensor_scalar_mul(tmpa[:, 512 * c:512 * c + 512], sa2[c][:], 1.0 / D)
        mua2 = pools.tile([1, N], F32, tag="w32", name="mua2")
        nc.vector.tensor_tensor(mua2[:], mua[:], mua[:], mybir.AluOpType.mult)
        nc.vector.tensor_tensor(tmpa[:], tmpa[:], mua2[:], mybir.AluOpType.subtract)
        nc.vector.tensor_scalar_add(tmpa[:], tmpa[:], EPS)
        nc.scalar.activation(tmpa[:], tmpa[:], mybir.ActivationFunctionType.Sqrt)
        nc.vector.reciprocal(rsa[:], tmpa[:])
        mua16 = io.tile([1, N], BF16, tag="mua16")
        rsa16 = io.tile([1, N], BF16, tag="rsa16")
        nc.scalar.copy(out=mua16[:], in_=mua[:])
        nc.scalar.copy(out=rsa16[:], in_=rsa[:])
        muar = io.tile([P, N], BF16, tag="mur", name="muar")
        rsar = io.tile([P, N], BF16, tag="rsr", name="rsar")
        for vec, rep in [(mua16, muar), (rsa16, rsar)]:
            for c in range(2):
                pt = psum.tile([P, 512], F32, tag="proj", name="rep")
                nc.tensor.matmul(pt[:], ones_row[:], vec[:, 512 * c:512 * c + 512],
                                 start=True, stop=True)
                nc.scalar.copy(out=rep[:, 512 * c:512 * c + 512], in_=pt[:])
        # prod = u * (LN_a(attn)*gamma+beta), in attnT layout
        for s in range(4):
            nc.vector.tensor_tensor(attnT[s][:], attnT[s][:], muar[:],
                                    mybir.AluOpType.subtract)
            nc.vector.tensor_tensor(attnT[s][:], attnT[s][:], rsar[:],
                                    mybir.AluOpType.mult)
            nc.vector.tensor_scalar(attnT[s][:], attnT[s][:],
                                    small["ga_col"][:, s:s + 1],
                                    small["bb_col"][:, s:s + 1],
                                    mybir.AluOpType.mult, mybir.AluOpType.add)
            nc.vector.tensor_tensor(attnT[s][:], attnT[s][:], projT[s][:],
                                    mybir.AluOpType.mult)

        # ---- output projection + residual ----
        for t in range(NT):
            po = psum.tile([P, 512], F32, tag="proj", name="outp")
            for s in range(4):
                nc.tensor.matmul(po[:], attnT[s][:, P * t:P * t + P], wo[s][:],
                                 start=(s == 0), stop=False)
            nc.tensor.matmul(po[:], ones_row[:], small["b_o_row"][:],
                             start=False, stop=True)
            xtile = pools.tile([P, D], F32, tag="w32", name="xtile")
            nc.sync.dma_start(xtile[:], d["xr"][P * t:P * t + P, :])
            ot = pools.tile([P, D], F32, tag="w32", name="ot")
            nc.vector.tensor_tensor(ot[:], po[:], xtile[:], mybir.AluOpType.add)
            nc.vector.tensor_scalar(ot[:], ot[:], small["padout_col"][:, t:t + 1],
                                    None, mybir.AluOpType.mult)
            nc.sync.dma_start(out_t[P * t:P * t + P, :], ot[:])

    nc.compile()
    return nc


def _prep_inputs(inputs):
    x = np.asarray(inputs["x"], dtype=np.float32)
    ts = np.asarray(inputs["timestamps"]).astype(np.int64)
    pad = np.asarray(inputs["pad_mask"]).astype(np.float32)
    uvqk = np.asarray(inputs["uvqk"], dtype=np.float32)
    W_o = np.asarray(inputs["W_o"], dtype=np.float32)
    b_o = np.asarray(inputs["b_o"], dtype=np.float32)
    gx = np.asarray(inputs["gamma_x"], dtype=np.float32)
    bx = np.asarray(inputs["beta_x"], dtype=np.float32)
    ga = np.asarray(inputs["gamma_a"], dtype=np.float32)
    ba = np.asarray(inputs["beta_a"], dtype=np.float32)
    ts_w = np.asarray(inputs["ts_w"], dtype=np.float32)
    pos_w = np.asarray(inputs["pos_w"], dtype=np.float32)

    tsq = np.concatenate([ts[:, 1:], ts[:, -1:]], axis=1)  # [B, N]
    far, kmin_g, kmax_g, k1min, k1max = _plan_chunks(ts, tsq)

    uvqk_g = (uvqk * gx[:, None]).astype(NPBF)
    bU = bx @ uvqk  # [E]
    bU_col = bU.reshape(E // P, P).T.copy()  # [P, E//P]
    bUv_rep = np.broadcast_to(bU[512:1024], (P, 512)).copy()
    ga_col = ga.reshape(4, P).T.copy()
    ba_col = ba.reshape(4, P).T.copy()

    # pos-bias tiles in [m, n] layout + per-chunk base constants
    widths = [N - P * r for r in range(NT)]
    offs = np.concatenate([[0], np.cumsum(widths)]).astype(int)
    posacc = np.zeros((P, int(offs[-1])), np.float32)
    nidx = np.arange(N)
    for r in range(NT):
        m = P * r + np.arange(P)[:, None]
        nn = nidx[None, P * r:]
        posacc[:, offs[r]:offs[r + 1]] = pos_w[nn - m + (N - 1)]
        posacc[:, offs[r]:offs[r] + P] += ts_w[kmin_g]
        if r < NT - 1:
            posacc[:, offs[r] + P:offs[r] + 2 * P] += ts_w[k1min]
    for (r, n0, n1, kmin, kmax) in far:
        posacc[:, offs[r] + n0 - P * r: offs[r] + n1 - P * r] += ts_w[kmin]
    posacc = posacc.astype(np.float16)

    per_core = []
    for b in range(B):
        per_core.append({
            "xT": np.ascontiguousarray(x[b].T).astype(NPBF),
            "xr": np.ascontiguousarray(x[b]),
            "tsq_rep": np.broadcast_to(tsq[b].astype(np.float32), (P, N)).copy(),
            "tsk_col": np.ascontiguousarray(ts[b].astype(np.float32).reshape(NT, P).T),
            "uvqk_g": uvqk_g, "bU_col": bU_col, "bUv_rep": bUv_rep,
            "W_o": W_o.astype(NPBF), "b_o_row": b_o.reshape(1, D).astype(NPBF),
            "ga_col": ga_col, "bb_col": ba_col,
            "vscale_col": np.ascontiguousarray(
                ((1.0 - pad[b]) / N).astype(np.float32).reshape(NT, P).T),
            "padout_col": np.ascontiguousarray(
                (1.0 - pad[b]).astype(np.float32).reshape(NT, P).T),
            "posacc": posacc,
        })
    return per_core, (far, kmin_g, kmax_g, k1min, k1max, ts_w)


def kernel(**inputs):
    from concourse.bass_utils import run_bass_kernel_spmd

    per_core, (far, kmin_g, kmax_g, k1min, k1max, ts_w) = _prep_inputs(inputs)
    key = (tuple(far), kmin_g, kmax_g, k1min, k1max, ts_w.tobytes())
    if key not in _cache:
        _cache.clear()
        _cache[key] = _build(ts_w, far, kmin_g, kmax_g, k1min, k1max)
    nc = _cache[key]
    res = run_bass_kernel_spmd(nc, per_core, list(range(B)))
    out = np.stack([res.results[b]["out"] for b in range(B)], axis=0)
    return out.astype(np.float32)


# revision 25
# speedup vs baseline: 2.0144x; 1.4190x over previous
"""HSTU block kernel for Trainium2, 8-core data-parallel over batch.

Key layout/scheduling choices:
  - All PE matmul operands are 16-bit (bf16/f16): 1 cycle/row with fp32 PSUM
    accumulation.  x ships as xT [D, N] bf16 (stats + proj rhs) and row-major
    f32 (+b_o folded in) for the residual.
  - The rel-bias ts_w[bucket(log dt)] reconstruction (threshold passes) is
    spread across DVE / Act / Pool: Act produces Sign(y-th) indicators that a
    fused DVE scalar_tensor_tensor accumulates (1 DVE op/pass), Pool runs an
    independent accumulator chain, DVE runs the rest.  Emitted早 so it
    overlaps the x-stats/projection phase on PE.
  - Causal masking is baked into the bias (-100 on sub-diagonal cells makes
    silu underflow to 0 in f16) -- no affine_select, no qksil memsets; the
    attn@v matmuls restrict columns to the causal region instead.
  - qk logits are produced transposed (LT [key m, query n]); the rel-bias is
    preloaded into PSUM via an f16 identity matmul so the qk matmul
    accumulates on top of it.
"""

import sys

sys.path.insert(0, "/opt/trn_rl_repo")

import numpy as np
import ml_dtypes

import concourse.bass as bass
import concourse.tile as tile
import concourse.mybir as mybir
from concourse import bacc
from concourse.masks import make_identity

B, N, D = 8, 1024, 512
H, DV, DQ = 8, 64, 64
E = 2 * H * DV + 2 * H * DQ  # 2048
EPS = 1e-5
P = 128
NT = N // P  # 8 row tiles
F32 = mybir.dt.float32
F16 = mybir.dt.float16
BF16 = mybir.dt.bfloat16
NPBF = np.dtype(ml_dtypes.bfloat16)
NEG = -100.0  # baked causal-mask bias: silu(x + NEG) == 0 in f16

_cache = {}


def _bucket(d):
    d = np.maximum(np.abs(d), 1).astype(np.float32)
    return np.clip((np.log(d) / 0.301).astype(np.int32), 0, 128)


def _plan_chunks(ts, tsq):
    """Uniform-across-batch k-ranges for the threshold passes."""
    far = []  # (r, n0, n1, kmin, kmax)
    for r in range(NT):
        n0 = P * (r + 2)
        while n0 < N:
            n1 = min(((n0 // 512) + 1) * 512, N)
            dmin = int((tsq[:, n0] - ts[:, P * r + P - 1]).min())
            dmax = int((tsq[:, n1 - 1] - ts[:, P * r]).max())
            far.append((r, n0, n1, int(_bucket(dmin)), int(_bucket(dmax))))
            n0 = n1
    # diag band: n in [128r, 128r+128), cells n >= m only
    dmin_g = int((tsq - ts).min())
    dmax_g = 0
    for r in range(NT):
        dmax_g = max(dmax_g, int((tsq[:, P * r + P - 1] - ts[:, P * r]).max()))
    kmin_g, kmax_g = int(_bucket(max(dmin_g, 0))), int(_bucket(dmax_g))
    # band1: n in [128(r+1), 128(r+2)) for r=0..6
    d1min = min(int((tsq[:, P * (r + 1)] - ts[:, P * r + P - 1]).min()) for r in range(NT - 1))
    d1max = max(int((tsq[:, P * (r + 2) - 1] - ts[:, P * r]).max()) for r in range(NT - 1))
    k1min, k1max = int(_bucket(max(d1min, 0))), int(_bucket(d1max))
    return far, kmin_g, kmax_g, k1min, k1max


def _assign_diag(kmin_g, kmax_g):
    """Split diag-band threshold passes across Pool / Act-assist / DVE."""
    kd = list(range(kmin_g + 1, kmax_g + 1))
    n_p = min(len(kd) // 3, 16)
    n_a = min(len(kd) * 3 // 10, 14)
    kp = kd[len(kd) - n_p:]
    rest = kd[:len(kd) - n_p]
    ka = rest[len(rest) - n_a:] if n_a else []
    kdve = rest[:len(rest) - n_a]
    return kp, ka, kdve


def _build(ts_w_np, far, kmin_g, kmax_g, k1min, k1max):
    nc = bacc.Bacc()
    d = {}
    for name, shape, dt_ in [
        ("xT", [D, N], BF16), ("xr", [N, D], F32), ("tsq_rep", [P, N], F32),
        ("ntsk_col", [P, NT], F32), ("uvqk_g", [D, E], BF16),
        ("bU_col", [P, E // P], F32), ("bUv_row", [1, DV * H], BF16),
        ("W_o", [D, D], BF16),
        ("ga_col", [P, 4], F32), ("bb_col", [P, 4], F32),
        ("vscale_col", [P, NT], F32), ("padout_col", [P, NT], F32),
        ("nth_col", [P, max(1, len(_assign_diag(kmin_g, kmax_g)[1]))], F32),
        ("posacc", [P, 4608], F16),
    ]:
        d[name] = nc.dram_tensor(name, shape, dt_, kind="ExternalInput")
    out_t = nc.dram_tensor("out", [N, D], F32, kind="ExternalOutput")

    widths = [N - P * r for r in range(NT)]
    offs = np.concatenate([[0], np.cumsum(widths)]).astype(int)
    tsw = ts_w_np.astype(np.float64)
    cks = [float(tsw[k] - tsw[k - 1]) for k in range(1, 129)]
    TH = 2.0 * 0.301  # y = ln(d^2) threshold scale
    kp_d, ka_d, kd_d = _assign_diag(kmin_g, kmax_g)
    AF = mybir.ActivationFunctionType
    OP = mybir.AluOpType

    from contextlib import ExitStack
    with tile.TileContext(nc) as tc, ExitStack() as ctx:
        io = ctx.enter_context(tc.tile_pool(name="io", bufs=1))
        pools = ctx.enter_context(tc.tile_pool(name="work", bufs=4))
        kpool = ctx.enter_context(tc.tile_pool(name="kpool", bufs=2))
        psum = ctx.enter_context(tc.tile_pool(name="psum", bufs=2, space="PSUM"))
        psqk = ctx.enter_context(tc.tile_pool(name="psqk", bufs=2, space="PSUM"))
        psmall = ctx.enter_context(tc.tile_pool(name="psmall", bufs=2, space="PSUM"))

        # ---- DMAs (bias-prep inputs first) ----
        tsq_rep = io.tile([P, N], F32, tag="tsqr")
        nc.sync.dma_start(tsq_rep[:], d["tsq_rep"][:])
        small = {}
        for nm, sh, dt_ in [("ntsk_col", [P, NT], F32), ("bU_col", [P, E // P], F32),
                            ("bUv_row", [1, DV * H], BF16),
                            ("ga_col", [P, 4], F32), ("bb_col", [P, 4], F32),
                            ("vscale_col", [P, NT], F32), ("padout_col", [P, NT], F32),
                            ("nth_col", [P, max(1, len(ka_d))], F32)]:
            small[nm] = io.tile(sh, dt_, tag=nm, name=nm)
            nc.sync.dma_start(small[nm][:], d[nm][:])
        xT = [io.tile([P, N], BF16, tag=f"xT{s}", name=f"xT{s}") for s in range(4)]
        for s in range(4):
            nc.sync.dma_start(xT[s][:], d["xT"][P * s:P * s + P, :])
        acc = [io.tile([P, widths[r]], F16, tag=f"acc{r}", name=f"acc{r}") for r in range(NT)]
        for r in range(NT):
            nc.sync.dma_start(acc[r][:], d["posacc"][:, offs[r]:offs[r + 1]])

        ident = io.tile([P, P], F16, tag="ident")
        make_identity(nc, ident[:])
        ones_col = io.tile([P, 1], BF16, tag="ones_col")
        nc.vector.memset(ones_col[:], 1.0)
        ones_row = io.tile([1, P], BF16, tag="ones_row")
        nc.vector.memset(ones_row[:], 1.0)

        # ---- rel-bias prep: y = ln((tsq - tsk)^2) per row tile, f16 ----
        yh = [io.tile([P, widths[r]], F16, tag=f"yh{r}", name=f"yh{r}") for r in range(NT)]
        ystack = io.tile([P, N], F16, tag="ystack")
        ystack2 = io.tile([P, N - P], F16, tag="ystack2")
        for r in range(NT):
            w = widths[r]
            db = pools.tile([P, N], F32, tag="w32", name="db")
            d2 = pools.tile([P, N], F32, tag="w32", name="d2")
            nc.scalar.activation(db[:, :w], tsq_rep[:, P * r:N], AF.Identity,
                                 bias=small["ntsk_col"][:, r:r + 1], scale=1.0)
            nc.vector.tensor_tensor(d2[:, :w], db[:, :w], db[:, :w], OP.mult)
            nc.scalar.activation(yh[r][:], d2[:, :w], AF.Ln)
            nc.vector.tensor_copy(out=ystack[:, P * r:P * r + P], in_=yh[r][:, 0:P])
            if r < NT - 1:
                nc.vector.tensor_copy(out=ystack2[:, P * r:P * r + P], in_=yh[r][:, P:2 * P])

        # ---- layernorm stats of x (over D, via ones-matmul on xT) ----
        s1p = [psmall.tile([1, 512], F32, tag="s1", name=f"s1p{c}") for c in range(2)]
        s2p = [psmall.tile([1, 512], F32, tag="s2", name=f"s2p{c}") for c in range(2)]
        for s in range(4):
            sq = pools.tile([P, N], BF16, tag="wb16", name="sq")
            nc.vector.tensor_tensor(sq[:], xT[s][:], xT[s][:], OP.mult)
            for c in range(2):
                nc.tensor.matmul(s1p[c][:], ones_col[:],
                                 xT[s][:, 512 * c:512 * c + 512],
                                 start=(s == 0), stop=(s == 3))
                nc.tensor.matmul(s2p[c][:], ones_col[:],
                                 sq[:, 512 * c:512 * c + 512],
                                 start=(s == 0), stop=(s == 3))
        mu = io.tile([1, N], BF16, tag="mu")
        rs = io.tile([1, N], BF16, tag="rs")
        tmp1 = pools.tile([1, N], BF16, tag="wsm", name="tmp1")
        for c in range(2):
            nc.vector.tensor_scalar_mul(mu[:, 512 * c:512 * c + 512], s1p[c][:], 1.0 / D)
            nc.vector.tensor_scalar_mul(tmp1[:, 512 * c:512 * c + 512], s2p[c][:], 1.0 / D)
        mu2 = pools.tile([1, N], BF16, tag="wsm", name="mu2")
        nc.vector.tensor_tensor(mu2[:], mu[:], mu[:], OP.mult)
        nc.vector.tensor_tensor(tmp1[:], tmp1[:], mu2[:], OP.subtract)
        nc.vector.tensor_scalar_add(tmp1[:], tmp1[:], EPS)
        nc.scalar.activation(tmp1[:], tmp1[:], AF.Sqrt)
        with nc.allow_low_precision(reason="bf16 rstd is plenty for 2e-2 tol"):
            nc.vector.reciprocal(rs[:], tmp1[:])

        # replicate mu, rs to [P, N] (bf16)
        mur = io.tile([P, N], BF16, tag="mur")
        rsr = io.tile([P, N], BF16, tag="rsr")
        for vec, rep in [(mu, mur), (rs, rsr)]:
            for c in range(2):
                pt = psum.tile([P, 512], F32, tag="proj", name="rep")
                nc.tensor.matmul(pt[:], ones_row[:], vec[:, 512 * c:512 * c + 512],
                                 start=True, stop=True)
                nc.scalar.copy(out=rep[:, 512 * c:512 * c + 512], in_=pt[:])

        # xn'T = (xT - mu) * rs  (in place, bf16)
        xnt = xT
        for s in range(4):
            nc.vector.tensor_tensor(xnt[s][:], xT[s][:], mur[:], OP.subtract)
            nc.vector.tensor_tensor(xnt[s][:], xnt[s][:], rsr[:], OP.mult)

        # ---- projT for q,k tiles (u deferred to after attention) ----
        projT = {}
        for t in range(8, 16):
            projT[t] = io.tile([P, N], F16, tag=f"pT{t}", name=f"pT{t}")
            uvs = []
            for s in range(4):
                u1 = pools.tile([P, P], BF16, tag="uvs", name="u1")
                nc.sync.dma_start(u1[:], d["uvqk_g"][P * s:P * s + P, P * t:P * t + P])
                uvs.append(u1)
            for c in range(2):
                pt = psum.tile([P, 512], F32, tag="proj")
                for s in range(4):
                    nc.tensor.matmul(pt[:], uvs[s][:],
                                     xnt[s][:, 512 * c:512 * c + 512],
                                     start=(s == 0), stop=(s == 3))
                nc.scalar.activation(projT[t][:, 512 * c:512 * c + 512], pt[:],
                                     AF.Silu, bias=small["bU_col"][:, t:t + 1],
                                     scale=1.0)
        # v row-major, silu + (1-pad)/N scale; bias row folded into the matmul
        vt = [io.tile([P, D], F16, tag=f"v{r}", name=f"v{r}") for r in range(NT)]
        uvv = []
        for s in range(4):
            u2 = pools.tile([P, 512], BF16, tag="uvv", name="u2")
            nc.sync.dma_start(u2[:], d["uvqk_g"][P * s:P * s + P, 512:1024])
            uvv.append(u2)
        for r in range(NT):
            pt = psum.tile([P, 512], F32, tag="proj")
            for s in range(4):
                nc.tensor.matmul(pt[:], xnt[s][:, P * r:P * r + P],
                                 uvv[s][:], start=(s == 0), stop=False)
            nc.tensor.matmul(pt[:], ones_row[:], small["bUv_row"][:],
                             start=False, stop=True)
            tmpv = pools.tile([P, D], F16, tag="wv16", name="tmpv")
            nc.scalar.activation(tmpv[:], pt[:], AF.Silu)
            nc.vector.tensor_scalar(vt[r][:], tmpv[:], small["vscale_col"][:, r:r + 1],
                                    None, OP.mult)

        # ---- rel-bias threshold passes ----
        # diag band: DVE-own passes (first writes dacc), Act-assisted (Sign ->
        # fused DVE mult-add), Pool chain into gacc.
        dacc = io.tile([P, N], F16, tag="dacc")
        first = True
        for k in kd_d:
            if first:
                nc.vector.tensor_scalar(dacc[:], ystack[:], float(TH * k), cks[k - 1],
                                        OP.is_ge, OP.mult)
                first = False
            else:
                t = kpool.tile([P, N], F16, tag="kt")
                nc.vector.tensor_scalar(t[:], ystack[:], float(TH * k), cks[k - 1],
                                        OP.is_ge, OP.mult)
                nc.vector.tensor_tensor(dacc[:], dacc[:], t[:], OP.add)
        for j, k in enumerate(ka_d):
            sg = kpool.tile([P, N], F16, tag="sg")
            nc.scalar.activation(sg[:], ystack[:], AF.Sign,
                                 bias=small["nth_col"][:, j:j + 1], scale=1.0)
            nc.vector.scalar_tensor_tensor(dacc[:], sg[:], cks[k - 1] * 0.5,
                                           dacc[:], OP.mult, OP.add)
        ystack32 = io.tile([P, N], F32, tag="ys32", name="ystack32")
        nc.gpsimd.tensor_copy(out=ystack32[:], in_=ystack[:])
        gacc = io.tile([P, N], F32, tag="gacc", name="gacc")
        firstp = True
        for k in kp_d:
            if firstp:
                nc.gpsimd.tensor_scalar(gacc[:], ystack32[:], float(TH * k), cks[k - 1],
                                        OP.is_ge, OP.mult)
                firstp = False
            else:
                tg = kpool.tile([P, N], F32, tag="ktg")
                nc.gpsimd.tensor_scalar(tg[:], ystack32[:], float(TH * k), cks[k - 1],
                                        OP.is_ge, OP.mult)
                nc.gpsimd.tensor_tensor(gacc[:], gacc[:], tg[:], OP.add)
        if firstp:
            nc.gpsimd.memset(gacc[:], 0.0)
        # band1 passes (DVE)
        dacc2 = io.tile([P, N - P], F16, tag="dacc2")
        first = True
        for k in range(k1min + 1, k1max + 1):
            if first:
                nc.vector.tensor_scalar(dacc2[:], ystack2[:], float(TH * k), cks[k - 1],
                                        OP.is_ge, OP.mult)
                first = False
            else:
                t = kpool.tile([P, N], F16, tag="kt")
                nc.vector.tensor_scalar(t[:, :N - P], ystack2[:], float(TH * k),
                                        cks[k - 1], OP.is_ge, OP.mult)
                nc.vector.tensor_tensor(dacc2[:], dacc2[:], t[:, :N - P], OP.add)
        if first:
            nc.vector.memset(dacc2[:], 0.0)
        # far chunk passes (DVE); acch[r] covers cols [2P, w) only
        acch = [io.tile([P, widths[r] - 2 * P], F16, tag=f"acch{r}", name=f"acch{r}")
                for r in range(NT - 2)]
        for (r, n0, n1, kmin, kmax) in far:
            a, b2 = n0 - P * r - 2 * P, n1 - P * r - 2 * P
            if kmax == kmin:
                nc.vector.memset(acch[r][:, a:b2], 0.0)
                continue
            for k in range(kmin + 1, kmax + 1):
                if k == kmin + 1:
                    nc.vector.tensor_scalar(acch[r][:, a:b2], yh[r][:, a + 2 * P:b2 + 2 * P],
                                            float(TH * k), cks[k - 1], OP.is_ge, OP.mult)
                else:
                    t = kpool.tile([P, N], F16, tag="kt")
                    nc.vector.tensor_scalar(t[:, :b2 - a], yh[r][:, a + 2 * P:b2 + 2 * P],
                                            float(TH * k), cks[k - 1], OP.is_ge, OP.mult)
                    nc.vector.tensor_tensor(acch[r][:, a:b2], acch[r][:, a:b2],
                                            t[:, :b2 - a], OP.add)
        # merge into acc
        gacc16 = io.tile([P, N], F16, tag="gacc16")
        nc.scalar.copy(out=gacc16[:], in_=gacc[:])
        for r in range(NT):
            if r < NT - 2:
                nc.vector.tensor_tensor(acc[r][:, 2 * P:], acc[r][:, 2 * P:],
                                        acch[r][:], OP.add)
            nc.vector.tensor_tensor(acc[r][:, 0:P], acc[r][:, 0:P],
                                    dacc[:, P * r:P * r + P], OP.add)
            nc.vector.tensor_tensor(acc[r][:, 0:P], acc[r][:, 0:P],
                                    gacc16[:, P * r:P * r + P], OP.add)
            if r < NT - 1:
                nc.vector.tensor_tensor(acc[r][:, P:2 * P], acc[r][:, P:2 * P],
                                        dacc2[:, P * r:P * r + P], OP.add)

        # ---- attention per head ----
        wo = [io.tile([P, D], BF16, tag=f"wo{s}", name=f"wo{s}") for s in range(4)]
        for s in range(4):
            nc.sync.dma_start(wo[s][:], d["W_o"][P * s:P * s + P, :])

        qksil = [io.tile([P, N], F16, tag=f"qs{r}", name=f"qs{r}") for r in range(NT)]
        attnT = [io.tile([P, N], BF16, tag=f"aT{t}", name=f"aT{t}") for t in range(4)]
        for h in range(H):
            qt = projT[8 + h // 2]
            kt = projT[12 + h // 2]
            pq = 64 * (h % 2)
            for r in range(NT):
                n0 = P * r
                while n0 < N:
                    n1 = min(((n0 // 512) + 1) * 512, N)
                    pt = psqk.tile([P, 512], F32, tag="qk")
                    cw = n1 - n0
                    nc.tensor.matmul(pt[:, :cw], ident[:],
                                     acc[r][:, n0 - P * r:n1 - P * r],
                                     start=True, stop=False)
                    nc.tensor.matmul(pt[:, :cw], kt[pq:pq + 64, P * r:P * r + P],
                                     qt[pq:pq + 64, n0:n1], start=False, stop=True)
                    nc.scalar.activation(qksil[r][:, n0:n1], pt[:, :cw], AF.Silu)
                    n0 = n1
            for c in range(2):
                pa = psqk.tile([P, 512], F32, tag="qk", name="av")
                nsub = min(NT, 4 * (c + 1))
                for r in range(nsub):
                    a = max(0, P * r - 512 * c)
                    nc.tensor.matmul(pa[:64, a:512], vt[r][:, 64 * h:64 * h + 64],
                                     qksil[r][:, 512 * c + a:512 * c + 512],
                                     start=(r == 0), stop=(r == nsub - 1))
                at = attnT[h // 2]
                nc.vector.tensor_copy(out=at[pq:pq + 64, 512 * c:512 * c + 512],
                                      in_=pa[:64, :])

        # ---- u projection (needed only for the final gating multiply) ----
        for t in range(4):
            projT[t] = io.tile([P, N], BF16, tag=f"pT{t}", name=f"pT{t}")
            uvs = []
            for s in range(4):
                u1 = pools.tile([P, P], BF16, tag="uvs", name="u1")
                nc.sync.dma_start(u1[:], d["uvqk_g"][P * s:P * s + P, P * t:P * t + P])
                uvs.append(u1)
            for c in range(2):
                pt = psum.tile([P, 512], F32, tag="proj")
                for s in range(4):
                    nc.tensor.matmul(pt[:], uvs[s][:],
                                     xnt[s][:, 512 * c:512 * c + 512],
                                     start=(s == 0), stop=(s == 3))
                nc.scalar.activation(projT[t][:, 512 * c:512 * c + 512], pt[:],
                                     AF.Silu, bias=small["bU_col"][:, t:t + 1],
                                     scale=1.0)

        # ---- layernorm of attn (over E=512, partition dim) ----
        sa1 = [psmall.tile([1, 512], F32, tag="s1", name=f"sa1{c}") for c in range(2)]
        sa2 = [psmall.tile([1, 512], F32, tag="s2", name=f"sa2{c}") for c in range(2)]
        for c in range(2):
            for s in range(4):
                nc.tensor.matmul(sa1[c][:], ones_col[:],
                                 attnT[s][:, 512 * c:512 * c + 512],
                                 start=(s == 0), stop=(s == 3))
            for s in range(4):
                sqa = pools.tile([P, 512], BF16, tag="wb16", name="sqa")
                nc.vector.tensor_tensor(sqa[:], attnT[s][:, 512 * c:512 * c + 512],
                                        attnT[s][:, 512 * c:512 * c + 512], OP.mult)
                nc.tensor.matmul(sa2[c][:], ones_col[:], sqa[:],
                                 start=(s == 0), stop=(s == 3))
        mua = io.tile([1, N], BF16, tag="mua")
        rsa = io.tile([1, N], BF16, tag="rsa")
        tmpa = pools.tile([1, N], BF16, tag="wsm", name="tmpa")
        for c in range(2):
            nc.vector.tensor_scalar_mul(mua[:, 512 * c:512 * c + 512], sa1[c][:], 1.0 / D)
            nc.vector.tensor_scalar_mul(tmpa[:, 512 * c:512 * c + 512], sa2[c][:], 1.0 / D)
        mua2 = pools.tile([1, N], BF16, tag="wsm", name="mua2")
        nc.vector.tensor_tensor(mua2[:], mua[:], mua[:], OP.mult)
        nc.vector.tensor_tensor(tmpa[:], tmpa[:], mua2[:], OP.subtract)
        nc.vector.tensor_scalar_add(tmpa[:], tmpa[:], EPS)
        nc.scalar.activation(tmpa[:], tmpa[:], AF.Sqrt)
        with nc.allow_low_precision(reason="bf16 rstd is plenty for 2e-2 tol"):
            nc.vector.reciprocal(rsa[:], tmpa[:])
        muar = io.tile([P, N], BF16, tag="mur", name="muar")
        rsar = io.tile([P, N], BF16, tag="rsr", name="rsar")
        for vec, rep in [(mua, muar), (rsa, rsar)]:
            for c in range(2):
                pt = psum.tile([P, 512], F32, tag="proj", name="rep")
                nc.tensor.matmul(pt[:], ones_row[:], vec[:, 512 * c:512 * c + 512],
                                 start=True, stop=True)
                nc.scalar.copy(out=rep[:, 512 * c:512 * c + 512], in_=pt[:])
        # prod = u * (LN_a(attn)*gamma+beta), in attnT layout
        for s in range(4):
            nc.vector.tensor_tensor(attnT[s][:], attnT[s][:], muar[:], OP.subtract)
            nc.vector.tensor_tensor(attnT[s][:], attnT[s][:], rsar[:], OP.mult)
            nc.vector.tensor_scalar(attnT[s][:], attnT[s][:],
                                    small["ga_col"][:, s:s + 1],
                                    small["bb_col"][:, s:s + 1],
                                    OP.mult, OP.add)
            nc.vector.tensor_tensor(attnT[s][:], attnT[s][:], projT[s][:], OP.mult)

        # ---- output projection + residual (b_o pre-folded into xr) ----
        for t in range(NT):
            po = psum.tile([P, 512], F32, tag="proj", name="outp")
            for s in range(4):
                nc.tensor.matmul(po[:], attnT[s][:, P * t:P * t + P], wo[s][:],
                                 start=(s == 0), stop=(s == 3))
            xtile = pools.tile([P, D], F32, tag="w32", name="xtile")
            nc.sync.dma_start(xtile[:], d["xr"][P * t:P * t + P, :])
            ot = pools.tile([P, D], F32, tag="w32", name="ot")
            nc.vector.tensor_tensor(ot[:], po[:], xtile[:], OP.add)
            nc.vector.tensor_scalar(ot[:], ot[:], small["padout_col"][:, t:t + 1],
                                    None, OP.mult)
            nc.sync.dma_start(out_t[P * t:P * t + P, :], ot[:])

    nc.compile()
    return nc


def _prep_inputs(inputs):
    x = np.asarray(inputs["x"], dtype=np.float32)
    ts = np.asarray(inputs["timestamps"]).astype(np.int64)
    pad = np.asarray(inputs["pad_mask"]).astype(np.float32)
    uvqk = np.asarray(inputs["uvqk"], dtype=np.float32)
    W_o = np.asarray(inputs["W_o"], dtype=np.float32)
    b_o = np.asarray(inputs["b_o"], dtype=np.float32)
    gx = np.asarray(inputs["gamma_x"], dtype=np.float32)
    bx = np.asarray(inputs["beta_x"], dtype=np.float32)
    ga = np.asarray(inputs["gamma_a"], dtype=np.float32)
    ba = np.asarray(inputs["beta_a"], dtype=np.float32)
    ts_w = np.asarray(inputs["ts_w"], dtype=np.float32)
    pos_w = np.asarray(inputs["pos_w"], dtype=np.float32)

    tsq = np.concatenate([ts[:, 1:], ts[:, -1:]], axis=1)  # [B, N]
    far, kmin_g, kmax_g, k1min, k1max = _plan_chunks(ts, tsq)
    kp_d, ka_d, kd_d = _assign_diag(kmin_g, kmax_g)
    tsw = ts_w.astype(np.float64)
    cks = [float(tsw[k] - tsw[k - 1]) for k in range(1, 129)]
    # Act-assisted Sign passes contribute ck/2 * (sign+1): fold the +ck/2
    # constant into the diag base.
    diag_base = float(ts_w[kmin_g]) + 0.5 * sum(cks[k - 1] for k in ka_d)

    uvqk_g = (uvqk * gx[:, None]).astype(NPBF)
    bU = bx @ uvqk  # [E]
    bU_col = bU.reshape(E // P, P).T.copy()  # [P, E//P]
    bUv_row = bU[512:1024].reshape(1, 512).astype(NPBF)
    ga_col = ga.reshape(4, P).T.copy()
    ba_col = ba.reshape(4, P).T.copy()

    # pos-bias tiles in [m, n] layout + per-chunk base constants
    widths = [N - P * r for r in range(NT)]
    offs = np.concatenate([[0], np.cumsum(widths)]).astype(int)
    posacc = np.zeros((P, int(offs[-1])), np.float32)
    nidx = np.arange(N)
    pidx = np.arange(P)[:, None]
    for r in range(NT):
        m = P * r + pidx
        nn = nidx[None, P * r:]
        posacc[:, offs[r]:offs[r + 1]] = pos_w[nn - m + (N - 1)]
        posacc[:, offs[r]:offs[r] + P] += diag_base
        if r < NT - 1:
            posacc[:, offs[r] + P:offs[r] + 2 * P] += ts_w[k1min]
        # causal mask baked in: sub-diagonal cells of the diag block get a
        # large negative bias so silu(qk + bias) underflows to 0 in f16
        sub = pidx > nidx[None, :P]
        posacc[:, offs[r]:offs[r] + P] = np.where(
            sub, NEG, posacc[:, offs[r]:offs[r] + P])
    for (r, n0, n1, kmin, kmax) in far:
        posacc[:, offs[r] + n0 - P * r: offs[r] + n1 - P * r] += ts_w[kmin]
    posacc = posacc.astype(np.float16)

    xr = x + b_o[None, None, :]  # residual rows with b_o folded in
    TH = 2.0 * 0.301
    nth = np.array([-TH * k for k in ka_d], np.float32) if ka_d else np.zeros(1, np.float32)
    nth_col = np.broadcast_to(nth[None, :], (P, len(nth))).copy()

    per_core = []
    for b in range(B):
        per_core.append({
            "xT": np.ascontiguousarray(x[b].T).astype(NPBF),
            "xr": np.ascontiguousarray(xr[b]),
            "tsq_rep": np.broadcast_to(tsq[b].astype(np.float32), (P, N)).copy(),
            "ntsk_col": np.ascontiguousarray((-ts[b]).astype(np.float32).reshape(NT, P).T),
            "uvqk_g": uvqk_g, "bU_col": bU_col, "bUv_row": bUv_row,
            "W_o": W_o.astype(NPBF),
            "ga_col": ga_col, "bb_col": ba_col,
            "vscale_col": np.ascontiguousarray(
                ((1.0 - pad[b]) / N).astype(np.float32).reshape(NT, P).T),
            "padout_col": np.ascontiguousarray(
                (1.0 - pad[b]).astype(np.float32).reshape(NT, P).T),
            "nth_col": nth_col,
            "posacc": posacc,
        })
    return per_core, (far, kmin_g, kmax_g, k1min, k1max, ts_w)


def kernel(**inputs):
    from concourse.bass_utils import run_bass_kernel_spmd

    per_core, (far, kmin_g, kmax_g, k1min, k1max, ts_w) = _prep_inputs(inputs)
    key = (tuple(far), kmin_g, kmax_g, k1min, k1max, ts_w.tobytes())
    if key not in _cache:
        _cache.clear()
        _cache[key] = _build(ts_w, far, kmin_g, kmax_g, k1min, k1max)
    nc = _cache[key]
    res = run_bass_kernel_spmd(nc, per_core, list(range(B)))
    out = np.stack([res.results[b]["out"] for b in range(B)], axis=0)
    return out.astype(np.float32)


# revision 28
# speedup vs baseline: 2.1357x; 1.0602x over previous
"""HSTU block kernel for Trainium2, 8-core data-parallel over batch.

Key layout/scheduling choices:
  - All PE matmul operands are 16-bit (bf16/f16): 1 cycle/row with fp32 PSUM
    accumulation.  x ships as xT [D, N] bf16 (stats + proj rhs) and row-major
    f32 (+b_o folded in) for the residual.
  - The rel-bias ts_w[bucket(log dt)] reconstruction: y = ln|dt| comes from
    two Act ops (Abs with per-partition bias, then Ln -> f16).  Threshold
    indicator tiles t_k = ck*[y >= th_k] are DVE tensor_scalar ops (4x f16
    mode); their SUM is accumulated on the PE via identity matmuls into PSUM
    together with the pos-bias seed, then copied back over acc.  A slice of
    passes runs on Pool (own accumulator) and a slice accumulates on DVE
    (TensorTensor) -- both folded into the same PSUM chain.
  - Causal masking is baked into the bias (-100 on sub-diagonal cells makes
    silu underflow to 0 in f16) -- no affine_select, no qksil memsets; the
    attn@v matmuls restrict columns to the causal region instead.
  - qk logits are produced transposed (LT [key m, query n]); the rel-bias is
    preloaded into PSUM via an f16 identity matmul so the qk matmul
    accumulates on top of it.  Row tiles r<=3 use 1024-wide PSUM tiles (one
    silu per row tile).
  - PSUM budget (8 banks): stats-stack 2 (four [1,512] accumulators live at
    partition offsets 0/32/64/96 of one bank tile), shared [P,512] pool 2
    (proj/qk/av/repl/out), [P,1024] pool 4 (bias chains + wide qk chunks).
"""

import sys

sys.path.insert(0, "/opt/trn_rl_repo")

import numpy as np
import ml_dtypes

import concourse.bass as bass
import concourse.tile as tile
import concourse.mybir as mybir
from concourse import bacc
from concourse.masks import make_identity

B, N, D = 8, 1024, 512
H, DV, DQ = 8, 64, 64
E = 2 * H * DV + 2 * H * DQ  # 2048
EPS = 1e-5
P = 128
NT = N // P  # 8 row tiles
F32 = mybir.dt.float32
F16 = mybir.dt.float16
BF16 = mybir.dt.bfloat16
NPBF = np.dtype(ml_dtypes.bfloat16)
NEG = -100.0  # baked causal-mask bias: silu(x + NEG) == 0 in f16

# threshold-pass distribution knobs
N_POOL_DIAG = 12   # diag passes on Pool (own chain)
N_DVETT_DIAG = 10  # diag passes accumulated on DVE (dacc)
N_DVETT_B1 = 8     # band1 passes accumulated on DVE (dacc2)

_cache = {}


def _bucket(d):
    d = np.maximum(np.abs(d), 1).astype(np.float32)
    return np.clip((np.log(d) / 0.301).astype(np.int32), 0, 128)


def _plan_chunks(ts, tsq):
    """Uniform-across-batch k-ranges for the threshold passes."""
    far = []  # (r, n0, n1, kmin, kmax)
    for r in range(NT):
        n0 = P * (r + 2)
        while n0 < N:
            n1 = min(((n0 // 512) + 1) * 512, N)
            dmin = int((tsq[:, n0] - ts[:, P * r + P - 1]).min())
            dmax = int((tsq[:, n1 - 1] - ts[:, P * r]).max())
            far.append((r, n0, n1, int(_bucket(dmin)), int(_bucket(dmax))))
            n0 = n1
    # diag band: n in [128r, 128r+128), cells n >= m only
    dmin_g = int((tsq - ts).min())
    dmax_g = 0
    for r in range(NT):
        dmax_g = max(dmax_g, int((tsq[:, P * r + P - 1] - ts[:, P * r]).max()))
    kmin_g, kmax_g = int(_bucket(max(dmin_g, 0))), int(_bucket(dmax_g))
    # band1: n in [128(r+1), 128(r+2)) for r=0..6
    d1min = min(int((tsq[:, P * (r + 1)] - ts[:, P * r + P - 1]).min()) for r in range(NT - 1))
    d1max = max(int((tsq[:, P * (r + 2) - 1] - ts[:, P * r]).max()) for r in range(NT - 1))
    k1min, k1max = int(_bucket(max(d1min, 0))), int(_bucket(d1max))
    return far, kmin_g, kmax_g, k1min, k1max


def _build(ts_w_np, far, kmin_g, kmax_g, k1min, k1max):
    nc = bacc.Bacc()
    d = {}
    for name, shape, dt_ in [
        ("xT", [D, N], BF16), ("xr", [N, D], F32), ("tsq_rep", [P, N], F32),
        ("ntsk_col", [P, NT], F32), ("uvqk_g", [D, E], BF16),
        ("bU_col", [P, E // P], F32), ("bUv_row", [1, DV * H], BF16),
        ("W_o", [D, D], BF16),
        ("ga_col", [P, 4], F32), ("bb_col", [P, 4], F32),
        ("vscale_col", [P, NT], F32), ("padout_col", [P, NT], F32),
        ("posacc", [P, 4608], F16),
    ]:
        d[name] = nc.dram_tensor(name, shape, dt_, kind="ExternalInput")
    out_t = nc.dram_tensor("out", [N, D], F32, kind="ExternalOutput")

    widths = [N - P * r for r in range(NT)]
    offs = np.concatenate([[0], np.cumsum(widths)]).astype(int)
    tsw = ts_w_np.astype(np.float64)
    cks = [float(tsw[k] - tsw[k - 1]) for k in range(1, 129)]
    TH = 0.301  # y = ln|d| threshold scale
    AF = mybir.ActivationFunctionType
    OP = mybir.AluOpType

    # pass assignment for the diag band
    kd_all = list(range(kmin_g + 1, kmax_g + 1))
    n_p = min(N_POOL_DIAG, len(kd_all))
    kp_d = kd_all[len(kd_all) - n_p:]
    rest = kd_all[:len(kd_all) - n_p]
    n_t = min(N_DVETT_DIAG, len(rest))
    kt_d = rest[len(rest) - n_t:]
    kpe_d = rest[:len(rest) - n_t]
    kb_all = list(range(k1min + 1, k1max + 1))
    n_tb = min(N_DVETT_B1, len(kb_all))
    kt_b = kb_all[len(kb_all) - n_tb:]
    kpe_b = kb_all[:len(kb_all) - n_tb]

    from contextlib import ExitStack
    with tile.TileContext(nc) as tc, ExitStack() as ctx:
        io = ctx.enter_context(tc.tile_pool(name="io", bufs=1))
        pools = ctx.enter_context(tc.tile_pool(name="work", bufs=4))
        kpool = ctx.enter_context(tc.tile_pool(name="kpool", bufs=6))
        kgpool = ctx.enter_context(tc.tile_pool(name="kgpool", bufs=2))
        pq512 = ctx.enter_context(tc.tile_pool(name="pq512", bufs=2, space="PSUM"))
        pw1024 = ctx.enter_context(tc.tile_pool(name="pw1024", bufs=2, space="PSUM"))
        pstat = ctx.enter_context(tc.tile_pool(name="pstat", bufs=2, space="PSUM"))

        # ---- DMAs (bias-prep inputs first) ----
        tsq_rep = io.tile([P, N], F32, tag="tsqr")
        nc.sync.dma_start(tsq_rep[:], d["tsq_rep"][:])
        small = {}
        for nm, sh, dt_ in [("ntsk_col", [P, NT], F32), ("bU_col", [P, E // P], F32),
                            ("bUv_row", [1, DV * H], BF16),
                            ("ga_col", [P, 4], F32), ("bb_col", [P, 4], F32),
                            ("vscale_col", [P, NT], F32), ("padout_col", [P, NT], F32)]:
            small[nm] = io.tile(sh, dt_, tag=nm, name=nm)
            nc.sync.dma_start(small[nm][:], d[nm][:])
        xT = [io.tile([P, N], BF16, tag=f"xT{s}", name=f"xT{s}") for s in range(4)]
        for s in range(4):
            nc.sync.dma_start(xT[s][:], d["xT"][P * s:P * s + P, :])
        acc = [io.tile([P, widths[r]], F16, tag=f"acc{r}", name=f"acc{r}") for r in range(NT)]
        for r in range(NT):
            nc.sync.dma_start(acc[r][:], d["posacc"][:, offs[r]:offs[r + 1]])

        ident = io.tile([P, P], F16, tag="ident")
        make_identity(nc, ident[:])
        ones_col = io.tile([P, 1], BF16, tag="ones_col")
        nc.vector.memset(ones_col[:], 1.0)
        ones_row = io.tile([1, P], BF16, tag="ones_row")
        nc.vector.memset(ones_row[:], 1.0)

        # ---- rel-bias prep: y = ln|tsq - tsk| per row tile, f16 (Act only) ----
        yh = [io.tile([P, widths[r]], F16, tag=f"yh{r}", name=f"yh{r}") for r in range(NT)]
        ystack = io.tile([P, N], F16, tag="ystack")
        ystack2 = io.tile([P, N - P], F16, tag="ystack2")
        for r in range(NT):
            w = widths[r]
            db = pools.tile([P, N], F32, tag="w32", name="db")
            nc.scalar.activation(db[:, :w], tsq_rep[:, P * r:N], AF.Abs,
                                 bias=small["ntsk_col"][:, r:r + 1], scale=1.0)
            nc.scalar.activation(yh[r][:], db[:, :w], AF.Ln)
            nc.vector.tensor_copy(out=ystack[:, P * r:P * r + P], in_=yh[r][:, 0:P])
            if r < NT - 1:
                nc.vector.tensor_copy(out=ystack2[:, P * r:P * r + P], in_=yh[r][:, P:2 * P])

        # ---- layernorm stats of x: four [1,512] accumulators stacked in one
        # PSUM bank at partition offsets 0/32/64/96 ----
        st1 = pstat.tile([P, 512], F32, tag="st", name="st_x")
        st1b = pstat.tile([P, 512], F32, tag="st", name="st_xb")
        srow = [st1[0:1, :], st1[32:33, :], st1[64:65, :], st1b[0:1, :]]
        for s in range(4):
            sq = pools.tile([P, N], BF16, tag="wb16", name="sq")
            nc.vector.tensor_tensor(sq[:], xT[s][:], xT[s][:], OP.mult)
            for c in range(2):
                nc.tensor.matmul(srow[c][:], ones_col[:],
                                 xT[s][:, 512 * c:512 * c + 512],
                                 start=(s == 0), stop=(s == 3))
                nc.tensor.matmul(srow[2 + c][:], ones_col[:],
                                 sq[:, 512 * c:512 * c + 512],
                                 start=(s == 0), stop=(s == 3))
        mu = io.tile([1, N], BF16, tag="mu")
        rs = io.tile([1, N], BF16, tag="rs")
        tmp1 = pools.tile([1, N], BF16, tag="wsm", name="tmp1")
        for c in range(2):
            nc.vector.tensor_scalar_mul(mu[:, 512 * c:512 * c + 512], srow[c][:], 1.0 / D)
            nc.vector.tensor_scalar_mul(tmp1[:, 512 * c:512 * c + 512], srow[2 + c][:], 1.0 / D)
        mu2 = pools.tile([1, N], BF16, tag="wsm", name="mu2")
        nc.vector.tensor_tensor(mu2[:], mu[:], mu[:], OP.mult)
        nc.vector.tensor_tensor(tmp1[:], tmp1[:], mu2[:], OP.subtract)
        nc.vector.tensor_scalar_add(tmp1[:], tmp1[:], EPS)
        nc.scalar.activation(tmp1[:], tmp1[:], AF.Sqrt)
        with nc.allow_low_precision(reason="bf16 rstd is plenty for 2e-2 tol"):
            nc.vector.reciprocal(rs[:], tmp1[:])

        # replicate mu, rs to [P, N] (bf16)
        mur = io.tile([P, N], BF16, tag="mur")
        rsr = io.tile([P, N], BF16, tag="rsr")
        for vec, rep in [(mu, mur), (rs, rsr)]:
            for c in range(2):
                pt = pq512.tile([P, 512], F32, tag="qk", name="rep")
                nc.tensor.matmul(pt[:], ones_row[:], vec[:, 512 * c:512 * c + 512],
                                 start=True, stop=True)
                nc.scalar.copy(out=rep[:, 512 * c:512 * c + 512], in_=pt[:])

        # xn'T = (xT - mu) * rs  (in place, bf16)
        xnt = xT
        for s in range(4):
            nc.vector.tensor_tensor(xnt[s][:], xT[s][:], mur[:], OP.subtract)
            nc.vector.tensor_tensor(xnt[s][:], xnt[s][:], rsr[:], OP.mult)

        # ---- DVE-accumulated threshold chains (into dacc / dacc2) ----
        dacc = io.tile([P, N], F16, tag="dacc")
        for i, k in enumerate(kt_d):
            if i == 0:
                nc.vector.tensor_scalar(dacc[:], ystack[:], float(TH * k), cks[k - 1],
                                        OP.is_ge, OP.mult)
            else:
                t = kpool.tile([P, N], F16, tag="kt")
                nc.vector.tensor_scalar(t[:], ystack[:], float(TH * k), cks[k - 1],
                                        OP.is_ge, OP.mult)
                nc.vector.tensor_tensor(dacc[:], dacc[:], t[:], OP.add)
        dacc2 = io.tile([P, N - P], F16, tag="dacc2")
        for i, k in enumerate(kt_b):
            if i == 0:
                nc.vector.tensor_scalar(dacc2[:], ystack2[:], float(TH * k), cks[k - 1],
                                        OP.is_ge, OP.mult)
            else:
                t = kpool.tile([P, N], F16, tag="kt")
                nc.vector.tensor_scalar(t[:, :N - P], ystack2[:], float(TH * k),
                                        cks[k - 1], OP.is_ge, OP.mult)
                nc.vector.tensor_tensor(dacc2[:], dacc2[:], t[:, :N - P], OP.add)

        # ---- Pool threshold chain (into gacc) ----
        ystack32 = io.tile([P, N], F32, tag="ys32", name="ystack32")
        nc.gpsimd.tensor_copy(out=ystack32[:], in_=ystack[:])
        gacc = io.tile([P, N], F32, tag="gacc", name="gacc")
        for i, k in enumerate(kp_d):
            if i == 0:
                nc.gpsimd.tensor_scalar(gacc[:], ystack32[:], float(TH * k), cks[k - 1],
                                        OP.is_ge, OP.mult)
            else:
                tg = kgpool.tile([P, N], F32, tag="ktg")
                nc.gpsimd.tensor_scalar(tg[:], ystack32[:], float(TH * k), cks[k - 1],
                                        OP.is_ge, OP.mult)
                nc.gpsimd.tensor_tensor(gacc[:], gacc[:], tg[:], OP.add)
        if not kp_d:
            nc.gpsimd.memset(gacc[:], 0.0)

        # ---- projections interleaved with the PE bias chain ----
        # diag chain accumulates in a [P,1024] PSUM tile: pos seeds + PE-path
        # t_k tiles + dacc + gacc16 folds, then copied back over acc[r][:,0:P].
        pbd = pw1024.tile([P, N], F32, tag="wide", name="bias_diag")

        def diag_seed():
            # start each 512-chunk with the first full-width accumulant
            pass

        # interleave emission: proj tile, then a slice of diag t_k matmuls
        diag_started = [False, False]

        def emit_diag_tk(k):
            t = kpool.tile([P, N], F16, tag="kt")
            nc.vector.tensor_scalar(t[:], ystack[:], float(TH * k), cks[k - 1],
                                    OP.is_ge, OP.mult)
            for c in range(2):
                nc.tensor.matmul(pbd[:, 512 * c:512 * c + 512], ident[:],
                                 t[:, 512 * c:512 * c + 512],
                                 start=(not diag_started[c]), stop=False)
                diag_started[c] = True

        projT = {}
        diag_iter = list(kpe_d)

        def emit_proj_tile(t_idx, dtype):
            projT[t_idx] = io.tile([P, N], dtype, tag=f"pT{t_idx}", name=f"pT{t_idx}")
            uvs = []
            for s in range(4):
                u1 = pools.tile([P, P], BF16, tag="uvs", name="u1")
                nc.sync.dma_start(u1[:], d["uvqk_g"][P * s:P * s + P,
                                                     P * t_idx:P * t_idx + P])
                uvs.append(u1)
            for c in range(2):
                pt = pq512.tile([P, 512], F32, tag="qk", name="proj")
                for s in range(4):
                    nc.tensor.matmul(pt[:], uvs[s][:],
                                     xnt[s][:, 512 * c:512 * c + 512],
                                     start=(s == 0), stop=(s == 3))
                nc.scalar.activation(projT[t_idx][:, 512 * c:512 * c + 512], pt[:],
                                     AF.Silu, bias=small["bU_col"][:, t_idx:t_idx + 1],
                                     scale=1.0)

        for t_idx in range(8, 16):
            emit_proj_tile(t_idx, F16)
            for _ in range(3):
                if diag_iter:
                    emit_diag_tk(diag_iter.pop(0))
        # v row-major, silu + (1-pad)/N scale; bias row folded into the matmul
        vt = [io.tile([P, D], F16, tag=f"v{r}", name=f"v{r}") for r in range(NT)]
        uvv = []
        for s in range(4):
            u2 = pools.tile([P, 512], BF16, tag="uvv", name="u2")
            nc.sync.dma_start(u2[:], d["uvqk_g"][P * s:P * s + P, 512:1024])
            uvv.append(u2)
        for r in range(NT):
            pt = pq512.tile([P, 512], F32, tag="qk", name="projv")
            for s in range(4):
                nc.tensor.matmul(pt[:], xnt[s][:, P * r:P * r + P],
                                 uvv[s][:], start=(s == 0), stop=False)
            nc.tensor.matmul(pt[:], ones_row[:], small["bUv_row"][:],
                             start=False, stop=True)
            tmpv = pools.tile([P, D], F16, tag="wv16", name="tmpv")
            nc.scalar.activation(tmpv[:], pt[:], AF.Silu)
            nc.vector.tensor_scalar(vt[r][:], tmpv[:], small["vscale_col"][:, r:r + 1],
                                    None, OP.mult)
            if diag_iter:
                emit_diag_tk(diag_iter.pop(0))
        while diag_iter:
            emit_diag_tk(diag_iter.pop(0))

        # fold gacc (via f16 copy) + dacc + pos seeds into the diag chain
        gacc16 = io.tile([P, N], F16, tag="gacc16")
        nc.scalar.copy(out=gacc16[:], in_=gacc[:])
        for c in range(2):
            if kt_d:
                nc.tensor.matmul(pbd[:, 512 * c:512 * c + 512], ident[:],
                                 dacc[:, 512 * c:512 * c + 512],
                                 start=(not diag_started[c]), stop=False)
                diag_started[c] = True
            if kp_d:
                nc.tensor.matmul(pbd[:, 512 * c:512 * c + 512], ident[:],
                                 gacc16[:, 512 * c:512 * c + 512],
                                 start=(not diag_started[c]), stop=False)
                diag_started[c] = True
        for r in range(NT):
            c = r // 4
            nc.tensor.matmul(pbd[:, P * r:P * r + P], ident[:], acc[r][:, 0:P],
                             start=(not diag_started[c]), stop=(r % 4 == 3))
            diag_started[c] = True
        for r in range(NT):
            nc.scalar.copy(out=acc[r][:, 0:P], in_=pbd[:, P * r:P * r + P])

        # band1 chain: [P, 896] in a wide PSUM tile
        pbb = pw1024.tile([P, N], F32, tag="wide", name="bias_b1")
        b1_started = [False, False]
        for k in kpe_b:
            t = kpool.tile([P, N], F16, tag="kt")
            nc.vector.tensor_scalar(t[:, :N - P], ystack2[:], float(TH * k), cks[k - 1],
                                    OP.is_ge, OP.mult)
            for c in range(2):
                w0, w1 = 512 * c, min(512 * c + 512, N - P)
                nc.tensor.matmul(pbb[:, w0:w1], ident[:], t[:, w0:w1],
                                 start=(not b1_started[c]), stop=False)
                b1_started[c] = True
        if kt_b:
            for c in range(2):
                w0, w1 = 512 * c, min(512 * c + 512, N - P)
                nc.tensor.matmul(pbb[:, w0:w1], ident[:], dacc2[:, w0:w1],
                                 start=(not b1_started[c]), stop=False)
                b1_started[c] = True
        for r in range(NT - 1):
            c = r // 4
            nc.tensor.matmul(pbb[:, P * r:P * r + P], ident[:], acc[r][:, P:2 * P],
                             start=(not b1_started[c]), stop=(r % 4 == 3 or r == NT - 2))
            b1_started[c] = True
        for r in range(NT - 1):
            nc.scalar.copy(out=acc[r][:, P:2 * P], in_=pbb[:, P * r:P * r + P])

        # far chunks: per-chunk PSUM accumulation (skip chunks with no passes)
        for (r, n0, n1, kmin, kmax) in far:
            if kmax == kmin:
                continue
            a, b2 = n0 - P * r, n1 - P * r
            w = b2 - a
            pf = pq512.tile([P, 512], F32, tag="qk", name="farc")
            for j, k in enumerate(range(kmin + 1, kmax + 1)):
                t = kpool.tile([P, N], F16, tag="kt")
                nc.vector.tensor_scalar(t[:, :w], yh[r][:, a:b2], float(TH * k),
                                        cks[k - 1], OP.is_ge, OP.mult)
                nc.tensor.matmul(pf[:, :w], ident[:], t[:, :w],
                                 start=(j == 0), stop=False)
            nc.tensor.matmul(pf[:, :w], ident[:], acc[r][:, a:b2],
                             start=False, stop=True)
            nc.scalar.copy(out=acc[r][:, a:b2], in_=pf[:, :w])

        # ---- attention per head ----
        wo = [io.tile([P, D], BF16, tag=f"wo{s}", name=f"wo{s}") for s in range(4)]
        for s in range(4):
            nc.sync.dma_start(wo[s][:], d["W_o"][P * s:P * s + P, :])

        qksil = [io.tile([P, N], F16, tag=f"qs{r}", name=f"qs{r}") for r in range(NT)]
        attnT = [io.tile([P, N], BF16, tag=f"aT{t}", name=f"aT{t}") for t in range(4)]
        for h in range(H):
            qt = projT[8 + h // 2]
            kt = projT[12 + h // 2]
            pq = 64 * (h % 2)
            for r in range(NT):
                n0 = P * r
                if r < 4:
                    # one wide PSUM tile for the whole row: [n0, 1024)
                    pt = pw1024.tile([P, N], F32, tag="wide", name="qkw")
                    m0 = n0
                    while m0 < N:
                        m1 = min(((m0 // 512) + 1) * 512, N)
                        nc.tensor.matmul(pt[:, m0:m1], ident[:],
                                         acc[r][:, m0 - n0:m1 - n0],
                                         start=True, stop=False)
                        nc.tensor.matmul(pt[:, m0:m1],
                                         kt[pq:pq + 64, P * r:P * r + P],
                                         qt[pq:pq + 64, m0:m1],
                                         start=False, stop=True)
                        m0 = m1
                    nc.scalar.activation(qksil[r][:, n0:N], pt[:, n0:N], AF.Silu)
                else:
                    pt = pq512.tile([P, 512], F32, tag="qk", name="qkn")
                    cw = N - n0
                    nc.tensor.matmul(pt[:, :cw], ident[:], acc[r][:],
                                     start=True, stop=False)
                    nc.tensor.matmul(pt[:, :cw], kt[pq:pq + 64, P * r:P * r + P],
                                     qt[pq:pq + 64, n0:N], start=False, stop=True)
                    nc.scalar.activation(qksil[r][:, n0:N], pt[:, :cw], AF.Silu)
            for c in range(2):
                pa = pq512.tile([P, 512], F32, tag="qk", name="av")
                nsub = min(NT, 4 * (c + 1))
                for r in range(nsub):
                    a = max(0, P * r - 512 * c)
                    nc.tensor.matmul(pa[:64, a:512], vt[r][:, 64 * h:64 * h + 64],
                                     qksil[r][:, 512 * c + a:512 * c + 512],
                                     start=(r == 0), stop=(r == nsub - 1))
                at = attnT[h // 2]
                nc.vector.tensor_copy(out=at[pq:pq + 64, 512 * c:512 * c + 512],
                                      in_=pa[:64, :])

        # ---- u projection (needed only for the final gating multiply) ----
        for t_idx in range(4):
            emit_proj_tile(t_idx, BF16)

        # ---- layernorm of attn (over E=512, partition dim) ----
        st2 = pstat.tile([P, 512], F32, tag="st", name="st_a")
        st2b = pstat.tile([P, 512], F32, tag="st", name="st_ab")
        arow = [st2[0:1, :], st2[32:33, :], st2[64:65, :], st2b[0:1, :]]
        for c in range(2):
            for s in range(4):
                nc.tensor.matmul(arow[c][:], ones_col[:],
                                 attnT[s][:, 512 * c:512 * c + 512],
                                 start=(s == 0), stop=(s == 3))
            for s in range(4):
                sqa = pools.tile([P, 512], BF16, tag="wb16", name="sqa")
                nc.vector.tensor_tensor(sqa[:], attnT[s][:, 512 * c:512 * c + 512],
                                        attnT[s][:, 512 * c:512 * c + 512], OP.mult)
                nc.tensor.matmul(arow[2 + c][:], ones_col[:], sqa[:],
                                 start=(s == 0), stop=(s == 3))
        mua = io.tile([1, N], BF16, tag="mua")
        rsa = io.tile([1, N], BF16, tag="rsa")
        tmpa = pools.tile([1, N], BF16, tag="wsm", name="tmpa")
        for c in range(2):
            nc.vector.tensor_scalar_mul(mua[:, 512 * c:512 * c + 512], arow[c][:], 1.0 / D)
            nc.vector.tensor_scalar_mul(tmpa[:, 512 * c:512 * c + 512], arow[2 + c][:], 1.0 / D)
        mua2 = pools.tile([1, N], BF16, tag="wsm", name="mua2")
        nc.vector.tensor_tensor(mua2[:], mua[:], mua[:], OP.mult)
        nc.vector.tensor_tensor(tmpa[:], tmpa[:], mua2[:], OP.subtract)
        nc.vector.tensor_scalar_add(tmpa[:], tmpa[:], EPS)
        nc.scalar.activation(tmpa[:], tmpa[:], AF.Sqrt)
        with nc.allow_low_precision(reason="bf16 rstd is plenty for 2e-2 tol"):
            nc.vector.reciprocal(rsa[:], tmpa[:])
        muar = io.tile([P, N], BF16, tag="mur", name="muar")
        rsar = io.tile([P, N], BF16, tag="rsr", name="rsar")
        for vec, rep in [(mua, muar), (rsa, rsar)]:
            for c in range(2):
                pt = pq512.tile([P, 512], F32, tag="qk", name="rep")
                nc.tensor.matmul(pt[:], ones_row[:], vec[:, 512 * c:512 * c + 512],
                                 start=True, stop=True)
                nc.scalar.copy(out=rep[:, 512 * c:512 * c + 512], in_=pt[:])
        # prod = u * (LN_a(attn)*gamma+beta), in attnT layout
        for s in range(4):
            nc.vector.tensor_tensor(attnT[s][:], attnT[s][:], muar[:], OP.subtract)
            nc.vector.tensor_tensor(attnT[s][:], attnT[s][:], rsar[:], OP.mult)
            nc.vector.tensor_scalar(attnT[s][:], attnT[s][:],
                                    small["ga_col"][:, s:s + 1],
                                    small["bb_col"][:, s:s + 1],
                                    OP.mult, OP.add)
            nc.vector.tensor_tensor(attnT[s][:], attnT[s][:], projT[s][:], OP.mult)

        # ---- output projection + residual (b_o pre-folded into xr) ----
        for t in range(NT):
            po = pq512.tile([P, 512], F32, tag="qk", name="outp")
            for s in range(4):
                nc.tensor.matmul(po[:], attnT[s][:, P * t:P * t + P], wo[s][:],
                                 start=(s == 0), stop=(s == 3))
            xtile = pools.tile([P, D], F32, tag="w32", name="xtile")
            nc.sync.dma_start(xtile[:], d["xr"][P * t:P * t + P, :])
            ot = pools.tile([P, D], F32, tag="w32", name="ot")
            nc.vector.tensor_tensor(ot[:], po[:], xtile[:], OP.add)
            nc.vector.tensor_scalar(ot[:], ot[:], small["padout_col"][:, t:t + 1],
                                    None, OP.mult)
            nc.sync.dma_start(out_t[P * t:P * t + P, :], ot[:])

    nc.compile()
    return nc


def _prep_inputs(inputs):
    x = np.asarray(inputs["x"], dtype=np.float32)
    ts = np.asarray(inputs["timestamps"]).astype(np.int64)
    pad = np.asarray(inputs["pad_mask"]).astype(np.float32)
    uvqk = np.asarray(inputs["uvqk"], dtype=np.float32)
    W_o = np.asarray(inputs["W_o"], dtype=np.float32)
    b_o = np.asarray(inputs["b_o"], dtype=np.float32)
    gx = np.asarray(inputs["gamma_x"], dtype=np.float32)
    bx = np.asarray(inputs["beta_x"], dtype=np.float32)
    ga = np.asarray(inputs["gamma_a"], dtype=np.float32)
    ba = np.asarray(inputs["beta_a"], dtype=np.float32)
    ts_w = np.asarray(inputs["ts_w"], dtype=np.float32)
    pos_w = np.asarray(inputs["pos_w"], dtype=np.float32)

    tsq = np.concatenate([ts[:, 1:], ts[:, -1:]], axis=1)  # [B, N]
    far, kmin_g, kmax_g, k1min, k1max = _plan_chunks(ts, tsq)

    uvqk_g = (uvqk * gx[:, None]).astype(NPBF)
    bU = bx @ uvqk  # [E]
    bU_col = bU.reshape(E // P, P).T.copy()  # [P, E//P]
    bUv_row = bU[512:1024].reshape(1, 512).astype(NPBF)
    ga_col = ga.reshape(4, P).T.copy()
    ba_col = ba.reshape(4, P).T.copy()

    # pos-bias tiles in [m, n] layout + per-chunk base constants
    widths = [N - P * r for r in range(NT)]
    offs = np.concatenate([[0], np.cumsum(widths)]).astype(int)
    posacc = np.zeros((P, int(offs[-1])), np.float32)
    nidx = np.arange(N)
    pidx = np.arange(P)[:, None]
    for r in range(NT):
        m = P * r + pidx
        nn = nidx[None, P * r:]
        posacc[:, offs[r]:offs[r + 1]] = pos_w[nn - m + (N - 1)]
        posacc[:, offs[r]:offs[r] + P] += ts_w[kmin_g]
        if r < NT - 1:
            posacc[:, offs[r] + P:offs[r] + 2 * P] += ts_w[k1min]
        # causal mask baked in: sub-diagonal cells of the diag block get a
        # large negative bias so silu(qk + bias) underflows to 0 in f16
        sub = pidx > nidx[None, :P]
        posacc[:, offs[r]:offs[r] + P] = np.where(
            sub, NEG, posacc[:, offs[r]:offs[r] + P])
    for (r, n0, n1, kmin, kmax) in far:
        posacc[:, offs[r] + n0 - P * r: offs[r] + n1 - P * r] += ts_w[kmin]
    posacc = posacc.astype(np.float16)

    xr = x + b_o[None, None, :]  # residual rows with b_o folded in

    per_core = []
    for b in range(B):
        per_core.append({
            "xT": np.ascontiguousarray(x[b].T).astype(NPBF),
            "xr": np.ascontiguousarray(xr[b]),
            "tsq_rep": np.broadcast_to(tsq[b].astype(np.float32), (P, N)).copy(),
            "ntsk_col": np.ascontiguousarray((-ts[b]).astype(np.float32).reshape(NT, P).T),
            "uvqk_g": uvqk_g, "bU_col": bU_col, "bUv_row": bUv_row,
            "W_o": W_o.astype(NPBF),
            "ga_col": ga_col, "bb_col": ba_col,
            "vscale_col": np.ascontiguousarray(
                ((1.0 - pad[b]) / N).astype(np.float32).reshape(NT, P).T),
            "padout_col": np.ascontiguousarray(
                (1.0 - pad[b]).astype(np.float32).reshape(NT, P).T),
            "posacc": posacc,
        })
    return per_core, (far, kmin_g, kmax_g, k1min, k1max, ts_w)


def kernel(**inputs):
    from concourse.bass_utils import run_bass_kernel_spmd

    per_core, (far, kmin_g, kmax_g, k1min, k1max, ts_w) = _prep_inputs(inputs)
    key = (tuple(far), kmin_g, kmax_g, k1min, k1max, ts_w.tobytes())
    if key not in _cache:
        _cache.clear()
        _cache[key] = _build(ts_w, far, kmin_g, kmax_g, k1min, k1max)
    nc = _cache[key]
    res = run_bass_kernel_spmd(nc, per_core, list(range(B)))
    out = np.stack([res.results[b]["out"] for b in range(B)], axis=0)
    return out.astype(np.float32)


# revision 29
# speedup vs baseline: 2.2273x; 1.0429x over previous
"""HSTU block kernel for Trainium2, 8-core data-parallel over batch.

Key layout/scheduling choices:
  - All PE matmul operands are 16-bit (bf16/f16): 1 cycle/row with fp32 PSUM
    accumulation.  x ships as xT [D, N] bf16 (stats + proj rhs) and row-major
    f32 (+b_o folded in) for the residual.
  - The rel-bias ts_w[bucket(log dt)] reconstruction: y = ln|dt| comes from
    two Act ops (Abs with per-partition bias, then Ln -> f16).  Threshold
    indicator tiles t_k = ck*[y >= th_k] are DVE tensor_scalar ops (4x f16
    mode); their SUM is accumulated on the PE via identity matmuls into PSUM
    together with the pos-bias seed, then copied back over acc.  A slice of
    passes runs on Pool (own accumulator) and a slice accumulates on DVE
    (TensorTensor) -- both folded into the same PSUM chain.
  - Causal masking is baked into the bias (-100 on sub-diagonal cells makes
    silu underflow to 0 in f16) -- no affine_select, no qksil memsets; the
    attn@v matmuls restrict columns to the causal region instead.
  - qk logits are produced transposed (LT [key m, query n]); the rel-bias is
    preloaded into PSUM via an f16 identity matmul so the qk matmul
    accumulates on top of it.  Row tiles r<=3 use 1024-wide PSUM tiles (one
    silu per row tile).
  - PSUM budget (8 banks): stats-stack 2 (four [1,512] accumulators live at
    partition offsets 0/32/64/96 of one bank tile), shared [P,512] pool 2
    (proj/qk/av/repl/out), [P,1024] pool 4 (bias chains + wide qk chunks).
"""

import sys

sys.path.insert(0, "/opt/trn_rl_repo")

import numpy as np
import ml_dtypes

import concourse.bass as bass
import concourse.tile as tile
import concourse.mybir as mybir
from concourse import bacc
from concourse.masks import make_identity

B, N, D = 8, 1024, 512
H, DV, DQ = 8, 64, 64
E = 2 * H * DV + 2 * H * DQ  # 2048
EPS = 1e-5
P = 128
NT = N // P  # 8 row tiles
F32 = mybir.dt.float32
F16 = mybir.dt.float16
BF16 = mybir.dt.bfloat16
NPBF = np.dtype(ml_dtypes.bfloat16)
NEG = -100.0  # baked causal-mask bias: silu(x + NEG) == 0 in f16

# threshold-pass distribution knobs
N_POOL_DIAG = 5    # diag passes on Pool (own chain)
N_DVETT_DIAG = 14  # diag passes accumulated on DVE (dacc)
N_DVETT_B1 = 8     # band1 passes accumulated on DVE (dacc2)

_cache = {}


def _bucket(d):
    d = np.maximum(np.abs(d), 1).astype(np.float32)
    return np.clip((np.log(d) / 0.301).astype(np.int32), 0, 128)


def _plan_chunks(ts, tsq):
    """Uniform-across-batch k-ranges for the threshold passes."""
    far = []  # (r, n0, n1, kmin, kmax)
    for r in range(NT):
        n0 = P * (r + 2)
        while n0 < N:
            n1 = min(((n0 // 512) + 1) * 512, N)
            dmin = int((tsq[:, n0] - ts[:, P * r + P - 1]).min())
            dmax = int((tsq[:, n1 - 1] - ts[:, P * r]).max())
            far.append((r, n0, n1, int(_bucket(dmin)), int(_bucket(dmax))))
            n0 = n1
    # diag band: n in [128r, 128r+128), cells n >= m only
    dmin_g = int((tsq - ts).min())
    dmax_g = 0
    for r in range(NT):
        dmax_g = max(dmax_g, int((tsq[:, P * r + P - 1] - ts[:, P * r]).max()))
    kmin_g, kmax_g = int(_bucket(max(dmin_g, 0))), int(_bucket(dmax_g))
    # band1: n in [128(r+1), 128(r+2)) for r=0..6
    d1min = min(int((tsq[:, P * (r + 1)] - ts[:, P * r + P - 1]).min()) for r in range(NT - 1))
    d1max = max(int((tsq[:, P * (r + 2) - 1] - ts[:, P * r]).max()) for r in range(NT - 1))
    k1min, k1max = int(_bucket(max(d1min, 0))), int(_bucket(d1max))
    return far, kmin_g, kmax_g, k1min, k1max


def _build(ts_w_np, far, kmin_g, kmax_g, k1min, k1max):
    nc = bacc.Bacc()
    d = {}
    for name, shape, dt_ in [
        ("xT", [D, N], BF16), ("xr", [N, D], F32), ("tsq_rep", [P, N], F32),
        ("ntsk_col", [P, NT], F32), ("uvqk_g", [D, E], BF16),
        ("bU_col", [P, E // P], F32), ("bUv_row", [1, DV * H], BF16),
        ("W_o", [D, D], BF16),
        ("ga_col", [P, 4], F32), ("bb_col", [P, 4], F32),
        ("vscale_col", [P, NT], F32), ("padout_col", [P, NT], F32),
        ("posacc", [P, 4608], F16),
    ]:
        d[name] = nc.dram_tensor(name, shape, dt_, kind="ExternalInput")
    out_t = nc.dram_tensor("out", [N, D], F32, kind="ExternalOutput")

    widths = [N - P * r for r in range(NT)]
    offs = np.concatenate([[0], np.cumsum(widths)]).astype(int)
    tsw = ts_w_np.astype(np.float64)
    cks = [float(tsw[k] - tsw[k - 1]) for k in range(1, 129)]
    TH = 0.301  # y = ln|d| threshold scale
    AF = mybir.ActivationFunctionType
    OP = mybir.AluOpType

    # pass assignment for the diag band
    kd_all = list(range(kmin_g + 1, kmax_g + 1))
    n_p = min(N_POOL_DIAG, len(kd_all))
    kp_d = kd_all[len(kd_all) - n_p:]
    rest = kd_all[:len(kd_all) - n_p]
    n_t = min(N_DVETT_DIAG, len(rest))
    kt_d = rest[len(rest) - n_t:]
    kpe_d = rest[:len(rest) - n_t]
    kb_all = list(range(k1min + 1, k1max + 1))
    n_tb = min(N_DVETT_B1, len(kb_all))
    kt_b = kb_all[len(kb_all) - n_tb:]
    kpe_b = kb_all[:len(kb_all) - n_tb]

    from contextlib import ExitStack
    with tile.TileContext(nc) as tc, ExitStack() as ctx:
        io = ctx.enter_context(tc.tile_pool(name="io", bufs=1))
        pools = ctx.enter_context(tc.tile_pool(name="work", bufs=4))
        kpool = ctx.enter_context(tc.tile_pool(name="kpool", bufs=6))
        kgpool = ctx.enter_context(tc.tile_pool(name="kgpool", bufs=2))
        pq512 = ctx.enter_context(tc.tile_pool(name="pq512", bufs=2, space="PSUM"))
        pw1024 = ctx.enter_context(tc.tile_pool(name="pw1024", bufs=2, space="PSUM"))
        pstat = ctx.enter_context(tc.tile_pool(name="pstat", bufs=2, space="PSUM"))

        # ---- DMAs (bias-prep inputs first) ----
        tsq_rep = io.tile([P, N], F32, tag="tsqr")
        nc.sync.dma_start(tsq_rep[:], d["tsq_rep"][:])
        small = {}
        for nm, sh, dt_ in [("ntsk_col", [P, NT], F32), ("bU_col", [P, E // P], F32),
                            ("bUv_row", [1, DV * H], BF16),
                            ("ga_col", [P, 4], F32), ("bb_col", [P, 4], F32),
                            ("vscale_col", [P, NT], F32), ("padout_col", [P, NT], F32)]:
            small[nm] = io.tile(sh, dt_, tag=nm, name=nm)
            nc.sync.dma_start(small[nm][:], d[nm][:])
        xT = [io.tile([P, N], BF16, tag=f"xT{s}", name=f"xT{s}") for s in range(4)]
        for s in range(4):
            nc.sync.dma_start(xT[s][:], d["xT"][P * s:P * s + P, :])
        acc = [io.tile([P, widths[r]], F16, tag=f"acc{r}", name=f"acc{r}") for r in range(NT)]
        for r in range(NT):
            nc.sync.dma_start(acc[r][:], d["posacc"][:, offs[r]:offs[r + 1]])

        ident = io.tile([P, P], F16, tag="ident")
        make_identity(nc, ident[:])
        ones_col = io.tile([P, 1], BF16, tag="ones_col")
        nc.vector.memset(ones_col[:], 1.0)
        ones_row = io.tile([1, P], BF16, tag="ones_row")
        nc.vector.memset(ones_row[:], 1.0)

        # ---- rel-bias prep: y = ln|tsq - tsk| per row tile, f16 (Act only) ----
        yh = [io.tile([P, widths[r]], F16, tag=f"yh{r}", name=f"yh{r}") for r in range(NT)]
        ystack = io.tile([P, N], F16, tag="ystack")
        ystack2 = io.tile([P, N - P], F16, tag="ystack2")
        for r in range(NT):
            w = widths[r]
            db = pools.tile([P, N], F32, tag="w32", name="db")
            nc.scalar.activation(db[:, :w], tsq_rep[:, P * r:N], AF.Abs,
                                 bias=small["ntsk_col"][:, r:r + 1], scale=1.0)
            nc.scalar.activation(yh[r][:], db[:, :w], AF.Ln)
            nc.vector.tensor_copy(out=ystack[:, P * r:P * r + P], in_=yh[r][:, 0:P])
            if r < NT - 1:
                nc.vector.tensor_copy(out=ystack2[:, P * r:P * r + P], in_=yh[r][:, P:2 * P])

        # ---- layernorm stats of x: four [1,512] accumulators stacked in one
        # PSUM bank at partition offsets 0/32/64/96 ----
        st1 = pstat.tile([P, 512], F32, tag="st", name="st_x")
        st1b = pstat.tile([P, 512], F32, tag="st", name="st_xb")
        srow = [st1[0:1, :], st1[32:33, :], st1[64:65, :], st1b[0:1, :]]
        for s in range(4):
            sq = pools.tile([P, N], BF16, tag="wb16", name="sq")
            nc.vector.tensor_tensor(sq[:], xT[s][:], xT[s][:], OP.mult)
            for c in range(2):
                nc.tensor.matmul(srow[c][:], ones_col[:],
                                 xT[s][:, 512 * c:512 * c + 512],
                                 start=(s == 0), stop=(s == 3))
                nc.tensor.matmul(srow[2 + c][:], ones_col[:],
                                 sq[:, 512 * c:512 * c + 512],
                                 start=(s == 0), stop=(s == 3))
        mu = io.tile([1, N], BF16, tag="mu")
        rs = io.tile([1, N], BF16, tag="rs")
        tmp1 = pools.tile([1, N], BF16, tag="wsm", name="tmp1")
        for c in range(2):
            nc.vector.tensor_scalar_mul(mu[:, 512 * c:512 * c + 512], srow[c][:], 1.0 / D)
            nc.vector.tensor_scalar_mul(tmp1[:, 512 * c:512 * c + 512], srow[2 + c][:], 1.0 / D)
        mu2 = pools.tile([1, N], BF16, tag="wsm", name="mu2")
        nc.vector.tensor_tensor(mu2[:], mu[:], mu[:], OP.mult)
        nc.vector.tensor_tensor(tmp1[:], tmp1[:], mu2[:], OP.subtract)
        nc.vector.tensor_scalar_add(tmp1[:], tmp1[:], EPS)
        nc.scalar.activation(tmp1[:], tmp1[:], AF.Sqrt)
        with nc.allow_low_precision(reason="bf16 rstd is plenty for 2e-2 tol"):
            nc.vector.reciprocal(rs[:], tmp1[:])

        # replicate mu, rs to [P, N] (bf16)
        mur = io.tile([P, N], BF16, tag="mur")
        rsr = io.tile([P, N], BF16, tag="rsr")
        for vec, rep in [(mu, mur), (rs, rsr)]:
            for c in range(2):
                pt = pq512.tile([P, 512], F32, tag="qk", name="rep")
                nc.tensor.matmul(pt[:], ones_row[:], vec[:, 512 * c:512 * c + 512],
                                 start=True, stop=True)
                nc.scalar.copy(out=rep[:, 512 * c:512 * c + 512], in_=pt[:])

        # xn'T = (xT - mu) * rs  (in place, bf16)
        xnt = xT
        for s in range(4):
            nc.vector.tensor_tensor(xnt[s][:], xT[s][:], mur[:], OP.subtract)
            nc.vector.tensor_tensor(xnt[s][:], xnt[s][:], rsr[:], OP.mult)

        # ---- DVE-accumulated threshold chains (into dacc / dacc2) ----
        dacc = io.tile([P, N], F16, tag="dacc")
        for i, k in enumerate(kt_d):
            if i == 0:
                nc.vector.tensor_scalar(dacc[:], ystack[:], float(TH * k), cks[k - 1],
                                        OP.is_ge, OP.mult)
            else:
                t = kpool.tile([P, N], F16, tag="kt")
                nc.vector.tensor_scalar(t[:], ystack[:], float(TH * k), cks[k - 1],
                                        OP.is_ge, OP.mult)
                nc.vector.tensor_tensor(dacc[:], dacc[:], t[:], OP.add)
        dacc2 = io.tile([P, N - P], F16, tag="dacc2")
        for i, k in enumerate(kt_b):
            if i == 0:
                nc.vector.tensor_scalar(dacc2[:], ystack2[:], float(TH * k), cks[k - 1],
                                        OP.is_ge, OP.mult)
            else:
                t = kpool.tile([P, N], F16, tag="kt")
                nc.vector.tensor_scalar(t[:, :N - P], ystack2[:], float(TH * k),
                                        cks[k - 1], OP.is_ge, OP.mult)
                nc.vector.tensor_tensor(dacc2[:], dacc2[:], t[:, :N - P], OP.add)

        # ---- Pool threshold chain (into gacc) ----
        ystack32 = io.tile([P, N], F32, tag="ys32", name="ystack32")
        nc.gpsimd.tensor_copy(out=ystack32[:], in_=ystack[:])
        gacc = io.tile([P, N], F32, tag="gacc", name="gacc")
        for i, k in enumerate(kp_d):
            if i == 0:
                nc.gpsimd.tensor_scalar(gacc[:], ystack32[:], float(TH * k), cks[k - 1],
                                        OP.is_ge, OP.mult)
            else:
                tg = kgpool.tile([P, N], F32, tag="ktg")
                nc.gpsimd.tensor_scalar(tg[:], ystack32[:], float(TH * k), cks[k - 1],
                                        OP.is_ge, OP.mult)
                nc.gpsimd.tensor_tensor(gacc[:], gacc[:], tg[:], OP.add)
        if not kp_d:
            nc.gpsimd.memset(gacc[:], 0.0)

        # ---- projections interleaved with the PE bias chain ----
        # diag chain accumulates in a [P,1024] PSUM tile: pos seeds + PE-path
        # t_k tiles + dacc + gacc16 folds, then copied back over acc[r][:,0:P].
        pbd = pw1024.tile([P, N], F32, tag="wide", name="bias_diag")

        def diag_seed():
            # start each 512-chunk with the first full-width accumulant
            pass

        # interleave emission: proj tile, then a slice of diag t_k matmuls
        diag_started = [False, False]

        def emit_diag_tk(k):
            t = kpool.tile([P, N], F16, tag="kt")
            nc.vector.tensor_scalar(t[:], ystack[:], float(TH * k), cks[k - 1],
                                    OP.is_ge, OP.mult)
            for c in range(2):
                nc.tensor.matmul(pbd[:, 512 * c:512 * c + 512], ident[:],
                                 t[:, 512 * c:512 * c + 512],
                                 start=(not diag_started[c]), stop=False)
                diag_started[c] = True

        projT = {}
        diag_iter = list(kpe_d)

        def emit_proj_tile(t_idx, dtype):
            projT[t_idx] = io.tile([P, N], dtype, tag=f"pT{t_idx}", name=f"pT{t_idx}")
            uvs = []
            for s in range(4):
                u1 = pools.tile([P, P], BF16, tag="uvs", name="u1")
                nc.sync.dma_start(u1[:], d["uvqk_g"][P * s:P * s + P,
                                                     P * t_idx:P * t_idx + P])
                uvs.append(u1)
            for c in range(2):
                pt = pq512.tile([P, 512], F32, tag="qk", name="proj")
                for s in range(4):
                    nc.tensor.matmul(pt[:], uvs[s][:],
                                     xnt[s][:, 512 * c:512 * c + 512],
                                     start=(s == 0), stop=(s == 3))
                nc.scalar.activation(projT[t_idx][:, 512 * c:512 * c + 512], pt[:],
                                     AF.Silu, bias=small["bU_col"][:, t_idx:t_idx + 1],
                                     scale=1.0)

        for t_idx in range(8, 16):
            emit_proj_tile(t_idx, F16)
            for _ in range(3):
                if diag_iter:
                    emit_diag_tk(diag_iter.pop(0))
        # v row-major, silu + (1-pad)/N scale; bias row folded into the matmul
        vt = [io.tile([P, D], F16, tag=f"v{r}", name=f"v{r}") for r in range(NT)]
        uvv = []
        for s in range(4):
            u2 = pools.tile([P, 512], BF16, tag="uvv", name="u2")
            nc.sync.dma_start(u2[:], d["uvqk_g"][P * s:P * s + P, 512:1024])
            uvv.append(u2)
        for r in range(NT):
            pt = pq512.tile([P, 512], F32, tag="qk", name="projv")
            for s in range(4):
                nc.tensor.matmul(pt[:], xnt[s][:, P * r:P * r + P],
                                 uvv[s][:], start=(s == 0), stop=False)
            nc.tensor.matmul(pt[:], ones_row[:], small["bUv_row"][:],
                             start=False, stop=True)
            tmpv = pools.tile([P, D], F16, tag="wv16", name="tmpv")
            nc.scalar.activation(tmpv[:], pt[:], AF.Silu)
            nc.vector.tensor_scalar(vt[r][:], tmpv[:], small["vscale_col"][:, r:r + 1],
                                    None, OP.mult)
            if diag_iter:
                emit_diag_tk(diag_iter.pop(0))
        while diag_iter:
            emit_diag_tk(diag_iter.pop(0))

        # u projection (consumed only at the final gating multiply, but PE/Act
        # have slack here)
        for t_idx in range(4):
            emit_proj_tile(t_idx, BF16)

        # fold gacc (via f16 copy) + dacc + pos seeds into the diag chain
        gacc16 = io.tile([P, N], F16, tag="gacc16")
        nc.scalar.copy(out=gacc16[:], in_=gacc[:])
        for c in range(2):
            if kt_d:
                nc.tensor.matmul(pbd[:, 512 * c:512 * c + 512], ident[:],
                                 dacc[:, 512 * c:512 * c + 512],
                                 start=(not diag_started[c]), stop=False)
                diag_started[c] = True
            if kp_d:
                nc.tensor.matmul(pbd[:, 512 * c:512 * c + 512], ident[:],
                                 gacc16[:, 512 * c:512 * c + 512],
                                 start=(not diag_started[c]), stop=False)
                diag_started[c] = True
        for r in range(NT):
            c = r // 4
            nc.tensor.matmul(pbd[:, P * r:P * r + P], ident[:], acc[r][:, 0:P],
                             start=(not diag_started[c]), stop=(r % 4 == 3))
            diag_started[c] = True
        for r in range(NT):
            nc.scalar.copy(out=acc[r][:, 0:P], in_=pbd[:, P * r:P * r + P])

        # band1 chain: [P, 896] in a wide PSUM tile
        pbb = pw1024.tile([P, N], F32, tag="wide", name="bias_b1")
        b1_started = [False, False]
        for k in kpe_b:
            t = kpool.tile([P, N], F16, tag="kt")
            nc.vector.tensor_scalar(t[:, :N - P], ystack2[:], float(TH * k), cks[k - 1],
                                    OP.is_ge, OP.mult)
            for c in range(2):
                w0, w1 = 512 * c, min(512 * c + 512, N - P)
                nc.tensor.matmul(pbb[:, w0:w1], ident[:], t[:, w0:w1],
                                 start=(not b1_started[c]), stop=False)
                b1_started[c] = True
        if kt_b:
            for c in range(2):
                w0, w1 = 512 * c, min(512 * c + 512, N - P)
                nc.tensor.matmul(pbb[:, w0:w1], ident[:], dacc2[:, w0:w1],
                                 start=(not b1_started[c]), stop=False)
                b1_started[c] = True
        for r in range(NT - 1):
            c = r // 4
            nc.tensor.matmul(pbb[:, P * r:P * r + P], ident[:], acc[r][:, P:2 * P],
                             start=(not b1_started[c]), stop=(r % 4 == 3 or r == NT - 2))
            b1_started[c] = True
        for r in range(NT - 1):
            nc.scalar.copy(out=acc[r][:, P:2 * P], in_=pbb[:, P * r:P * r + P])

        # far chunks: per-chunk PSUM accumulation (skip chunks with no passes)
        for (r, n0, n1, kmin, kmax) in far:
            if kmax == kmin:
                continue
            a, b2 = n0 - P * r, n1 - P * r
            w = b2 - a
            pf = pq512.tile([P, 512], F32, tag="qk", name="farc")
            for j, k in enumerate(range(kmin + 1, kmax + 1)):
                t = kpool.tile([P, N], F16, tag="kt")
                nc.vector.tensor_scalar(t[:, :w], yh[r][:, a:b2], float(TH * k),
                                        cks[k - 1], OP.is_ge, OP.mult)
                nc.tensor.matmul(pf[:, :w], ident[:], t[:, :w],
                                 start=(j == 0), stop=False)
            nc.tensor.matmul(pf[:, :w], ident[:], acc[r][:, a:b2],
                             start=False, stop=True)
            nc.scalar.copy(out=acc[r][:, a:b2], in_=pf[:, :w])

        # ---- attention per head ----
        wo = [io.tile([P, D], BF16, tag=f"wo{s}", name=f"wo{s}") for s in range(4)]
        for s in range(4):
            nc.sync.dma_start(wo[s][:], d["W_o"][P * s:P * s + P, :])

        qksil = [io.tile([P, N], F16, tag=f"qs{r}", name=f"qs{r}") for r in range(NT)]
        attnT = [io.tile([P, N], BF16, tag=f"aT{t}", name=f"aT{t}") for t in range(4)]
        for h in range(H):
            qt = projT[8 + h // 2]
            kt = projT[12 + h // 2]
            pq = 64 * (h % 2)
            for r in range(NT):
                n0 = P * r
                if r < 4:
                    # one wide PSUM tile for the whole row: [n0, 1024)
                    pt = pw1024.tile([P, N], F32, tag="wide", name="qkw")
                    m0 = n0
                    while m0 < N:
                        m1 = min(((m0 // 512) + 1) * 512, N)
                        nc.tensor.matmul(pt[:, m0:m1], ident[:],
                                         acc[r][:, m0 - n0:m1 - n0],
                                         start=True, stop=False)
                        nc.tensor.matmul(pt[:, m0:m1],
                                         kt[pq:pq + 64, P * r:P * r + P],
                                         qt[pq:pq + 64, m0:m1],
                                         start=False, stop=True)
                        m0 = m1
                    nc.scalar.activation(qksil[r][:, n0:N], pt[:, n0:N], AF.Silu)
                else:
                    pt = pq512.tile([P, 512], F32, tag="qk", name="qkn")
                    cw = N - n0
                    nc.tensor.matmul(pt[:, :cw], ident[:], acc[r][:],
                                     start=True, stop=False)
                    nc.tensor.matmul(pt[:, :cw], kt[pq:pq + 64, P * r:P * r + P],
                                     qt[pq:pq + 64, n0:N], start=False, stop=True)
                    nc.scalar.activation(qksil[r][:, n0:N], pt[:, :cw], AF.Silu)
            for c in range(2):
                pa = pq512.tile([P, 512], F32, tag="qk", name="av")
                nsub = min(NT, 4 * (c + 1))
                for r in range(nsub):
                    a = max(0, P * r - 512 * c)
                    nc.tensor.matmul(pa[:64, a:512], vt[r][:, 64 * h:64 * h + 64],
                                     qksil[r][:, 512 * c + a:512 * c + 512],
                                     start=(r == 0), stop=(r == nsub - 1))
                at = attnT[h // 2]
                nc.vector.tensor_copy(out=at[pq:pq + 64, 512 * c:512 * c + 512],
                                      in_=pa[:64, :])

        # ---- layernorm of attn (over E=512, partition dim) ----
        st2 = pstat.tile([P, 512], F32, tag="st", name="st_a")
        st2b = pstat.tile([P, 512], F32, tag="st", name="st_ab")
        arow = [st2[0:1, :], st2[32:33, :], st2[64:65, :], st2b[0:1, :]]
        for c in range(2):
            for s in range(4):
                nc.tensor.matmul(arow[c][:], ones_col[:],
                                 attnT[s][:, 512 * c:512 * c + 512],
                                 start=(s == 0), stop=(s == 3))
            for s in range(4):
                sqa = pools.tile([P, 512], BF16, tag="wb16", name="sqa")
                nc.vector.tensor_tensor(sqa[:], attnT[s][:, 512 * c:512 * c + 512],
                                        attnT[s][:, 512 * c:512 * c + 512], OP.mult)
                nc.tensor.matmul(arow[2 + c][:], ones_col[:], sqa[:],
                                 start=(s == 0), stop=(s == 3))
        mua = io.tile([1, N], BF16, tag="mua")
        rsa = io.tile([1, N], BF16, tag="rsa")
        tmpa = pools.tile([1, N], BF16, tag="wsm", name="tmpa")
        for c in range(2):
            nc.vector.tensor_scalar_mul(mua[:, 512 * c:512 * c + 512], arow[c][:], 1.0 / D)
            nc.vector.tensor_scalar_mul(tmpa[:, 512 * c:512 * c + 512], arow[2 + c][:], 1.0 / D)
        mua2 = pools.tile([1, N], BF16, tag="wsm", name="mua2")
        nc.vector.tensor_tensor(mua2[:], mua[:], mua[:], OP.mult)
        nc.vector.tensor_tensor(tmpa[:], tmpa[:], mua2[:], OP.subtract)
        nc.vector.tensor_scalar_add(tmpa[:], tmpa[:], EPS)
        nc.scalar.activation(tmpa[:], tmpa[:], AF.Sqrt)
        with nc.allow_low_precision(reason="bf16 rstd is plenty for 2e-2 tol"):
            nc.vector.reciprocal(rsa[:], tmpa[:])
        muar = io.tile([P, N], BF16, tag="mur", name="muar")
        rsar = io.tile([P, N], BF16, tag="rsr", name="rsar")
        for vec, rep in [(mua, muar), (rsa, rsar)]:
            for c in range(2):
                pt = pq512.tile([P, 512], F32, tag="qk", name="rep")
                nc.tensor.matmul(pt[:], ones_row[:], vec[:, 512 * c:512 * c + 512],
                                 start=True, stop=True)
                nc.scalar.copy(out=rep[:, 512 * c:512 * c + 512], in_=pt[:])
        # prod = u * (LN_a(attn)*gamma+beta), in attnT layout
        for s in range(4):
            nc.vector.tensor_tensor(attnT[s][:], attnT[s][:], muar[:], OP.subtract)
            nc.vector.tensor_tensor(attnT[s][:], attnT[s][:], rsar[:], OP.mult)
            nc.vector.tensor_scalar(attnT[s][:], attnT[s][:],
                                    small["ga_col"][:, s:s + 1],
                                    small["bb_col"][:, s:s + 1],
                                    OP.mult, OP.add)
            nc.vector.tensor_tensor(attnT[s][:], attnT[s][:], projT[s][:], OP.mult)

        # ---- output projection + residual (b_o pre-folded into xr) ----
        for t in range(NT):
            po = pq512.tile([P, 512], F32, tag="qk", name="outp")
            for s in range(4):
                nc.tensor.matmul(po[:], attnT[s][:, P * t:P * t + P], wo[s][:],
                                 start=(s == 0), stop=(s == 3))
            xtile = pools.tile([P, D], F32, tag="w32", name="xtile")
            nc.sync.dma_start(xtile[:], d["xr"][P * t:P * t + P, :])
            ot = pools.tile([P, D], F32, tag="w32", name="ot")
            nc.vector.tensor_tensor(ot[:], po[:], xtile[:], OP.add)
            nc.vector.tensor_scalar(ot[:], ot[:], small["padout_col"][:, t:t + 1],
                                    None, OP.mult)
            nc.sync.dma_start(out_t[P * t:P * t + P, :], ot[:])

    nc.compile()
    return nc


def _prep_inputs(inputs):
    x = np.asarray(inputs["x"], dtype=np.float32)
    ts = np.asarray(inputs["timestamps"]).astype(np.int64)
    pad = np.asarray(inputs["pad_mask"]).astype(np.float32)
    uvqk = np.asarray(inputs["uvqk"], dtype=np.float32)
    W_o = np.asarray(inputs["W_o"], dtype=np.float32)
    b_o = np.asarray(inputs["b_o"], dtype=np.float32)
    gx = np.asarray(inputs["gamma_x"], dtype=np.float32)
    bx = np.asarray(inputs["beta_x"], dtype=np.float32)
    ga = np.asarray(inputs["gamma_a"], dtype=np.float32)
    ba = np.asarray(inputs["beta_a"], dtype=np.float32)
    ts_w = np.asarray(inputs["ts_w"], dtype=np.float32)
    pos_w = np.asarray(inputs["pos_w"], dtype=np.float32)

    tsq = np.concatenate([ts[:, 1:], ts[:, -1:]], axis=1)  # [B, N]
    far, kmin_g, kmax_g, k1min, k1max = _plan_chunks(ts, tsq)

    uvqk_g = (uvqk * gx[:, None]).astype(NPBF)
    bU = bx @ uvqk  # [E]
    bU_col = bU.reshape(E // P, P).T.copy()  # [P, E//P]
    bUv_row = bU[512:1024].reshape(1, 512).astype(NPBF)
    ga_col = ga.reshape(4, P).T.copy()
    ba_col = ba.reshape(4, P).T.copy()

    # pos-bias tiles in [m, n] layout + per-chunk base constants
    widths = [N - P * r for r in range(NT)]
    offs = np.concatenate([[0], np.cumsum(widths)]).astype(int)
    posacc = np.zeros((P, int(offs[-1])), np.float32)
    nidx = np.arange(N)
    pidx = np.arange(P)[:, None]
    for r in range(NT):
        m = P * r + pidx
        nn = nidx[None, P * r:]
        posacc[:, offs[r]:offs[r + 1]] = pos_w[nn - m + (N - 1)]
        posacc[:, offs[r]:offs[r] + P] += ts_w[kmin_g]
        if r < NT - 1:
            posacc[:, offs[r] + P:offs[r] + 2 * P] += ts_w[k1min]
        # causal mask baked in: sub-diagonal cells of the diag block get a
        # large negative bias so silu(qk + bias) underflows to 0 in f16
        sub = pidx > nidx[None, :P]
        posacc[:, offs[r]:offs[r] + P] = np.where(
            sub, NEG, posacc[:, offs[r]:offs[r] + P])
    for (r, n0, n1, kmin, kmax) in far:
        posacc[:, offs[r] + n0 - P * r: offs[r] + n1 - P * r] += ts_w[kmin]
    posacc = posacc.astype(np.float16)

    xr = x + b_o[None, None, :]  # residual rows with b_o folded in

    per_core = []
    for b in range(B):
        per_core.append({
            "xT": np.ascontiguousarray(x[b].T).astype(NPBF),
            "xr": np.ascontiguousarray(xr[b]),
            "tsq_rep": np.broadcast_to(tsq[b].astype(np.float32), (P, N)).copy(),
            "ntsk_col": np.ascontiguousarray((-ts[b]).astype(np.float32).reshape(NT, P).T),
            "uvqk_g": uvqk_g, "bU_col": bU_col, "bUv_row": bUv_row,
            "W_o": W_o.astype(NPBF),
            "ga_col": ga_col, "bb_col": ba_col,
            "vscale_col": np.ascontiguousarray(
                ((1.0 - pad[b]) / N).astype(np.float32).reshape(NT, P).T),
            "padout_col": np.ascontiguousarray(
                (1.0 - pad[b]).astype(np.float32).reshape(NT, P).T),
            "posacc": posacc,
        })
    return per_core, (far, kmin_g, kmax_g, k1min, k1max, ts_w)


def kernel(**inputs):
    from concourse.bass_utils import run_bass_kernel_spmd

    per_core, (far, kmin_g, kmax_g, k1min, k1max, ts_w) = _prep_inputs(inputs)
    key = (tuple(far), kmin_g, kmax_g, k1min, k1max, ts_w.tobytes())
    if key not in _cache:
        _cache.clear()
        _cache[key] = _build(ts_w, far, kmin_g, kmax_g, k1min, k1max)
    nc = _cache[key]
    res = run_bass_kernel_spmd(nc, per_core, list(range(B)))
    out = np.stack([res.results[b]["out"] for b in range(B)], axis=0)
    return out.astype(np.float32)
